# revision 49
# baseline (speedup 1.0000x reference)
"""GroupedQueryAttention Trainium2 Bass kernel (v6).

Sharding: 8 cores = (B=2) x (G=4 KV groups). Each core computes, for its
(batch b, kv-group g): the 4 query heads' Q/K/V projections, causal flash
attention, and a partial output projection Y^T_g (bf16). Host sums the 4
partials per batch and adds an adjusted bias (bo + bv-term folded in).

Key structure (all transposed: token dim T on the free axis):
  xT[d, t]     uploaded pre-transposed from host (bf16)
  Q^T, K^T     from projection matmuls (W chunk stationary, xT moving)
  V^T -> V     PE transpose per 128-block, staged in the st PSUM slots
  S^T[s, t]  = (K^T s-block).T @ Q^T        (one 128-wide matmul per s-block)
  P^T        = exp(scale * S^T + mask)      (ACT, PSUM -> SBUF, bf16)
  O^T[dh, t] += (V s-block).T @ P^T         (PSUM accumulation over s-blocks)
  rowsum     += ones.T-style P sums (DVE bf16 adds per pair)
  Y^T[dm, t] = sum_c (Wo chunk).T @ O^T_c   (per 128-row dm block, bf16 out)

The V bias never enters the kernel: O = (P@(V0+1*bv))/rowsum = P@V0/rowsum
+ bv, and the constant bv contribution to Y is folded into bo on the host.

v6 scheduling (on the v2/v3 spine):
  - Unified filler stream: flash(tau) absorbs oproj(tau-1) m-blocks AND
    the K/V/Q projection chains of tau+1, so the tau boundary has no proj
    bubble. The V chain is split into two units - matmuls+copy early, PE
    transposes several steps later - so the transpose LDWEIGHTS never
    parks the PE FIFO on the ACT queue.
  - qt/kt PSUM->SBUF bias-copies on DVE (tensor_scalar_add), off the ACT
    FIFO that feeds exp.
  - pend FIFO depth 3 (PV consumes 3 pairs behind the S/exp front) to ride
    out exp latency spikes.
  - Light oproj budgets shift some m-blocks from the PE-bound flash(1/2)
    windows into the exp-bound flash(3); yT stores alternate the sync and
    gpsimd queues, and tail ys copies alternate ACT/DVE, so the drain ends
    within ~2us of the last matmul.

Normalize chain: ptsum adds (DVE, bf16) -> gpsimd 128-way all-reduce (f32)
-> reciprocal_approx_fast (DVE) -> mul (DVE), recip+mul deferred a few
consume slots so the DVE FIFO never blocks on the all-reduce.
"""

import sys

sys.path.insert(0, "/opt/trn_rl_repo")

from contextlib import ExitStack

import ml_dtypes
import numpy as np

import concourse.bass as bass  # noqa: F401
import concourse.tile as tile
from concourse import bacc, bass_isa, mybir
from concourse.bass_utils import run_bass_kernel_spmd

F32 = mybir.dt.float32
BF16 = mybir.dt.bfloat16
AF = mybir.ActivationFunctionType

D = 2048          # model dim
T = 2048          # tokens
DH = 128          # head dim
G = 4             # kv groups
HPG = 4           # query heads per group
QC = HPG * DH     # query cols per group = 512
ND = D // 128     # 16 contraction chunks
NTAU = 4          # t tiles of 512
TW = 512          # t tile width
MAXP = 2 * NTAU   # max pairs per head (tau=3)
SCALE = DH ** -0.5

TRACE = False
TRACE_KW = {}
LAST_RESULTS = None

_CACHE = {}


def _body(ctx, tc, xT, wq, wk, wv, wo, bq, bk, maskTd, identd, yT):
    nc = tc.nc

    # PSUM (16KB/partition exactly): acc 2x2KB + st-pair 2x4KB (shared with
    # V-transpose staging) + ot 2x2KB
    psacc = ctx.enter_context(tc.tile_pool(name="psacc", bufs=2, space="PSUM"))
    psst = ctx.enter_context(tc.tile_pool(name="psst", bufs=2, space="PSUM"))
    psot = ctx.enter_context(tc.tile_pool(name="psot", bufs=2, space="PSUM"))

    consts = ctx.enter_context(tc.tile_pool(name="consts", bufs=1))
    qkv = ctx.enter_context(tc.tile_pool(name="qkv", bufs=1))
    xtp = ctx.enter_context(tc.tile_pool(name="xtp", bufs=ND))
    wkp = ctx.enter_context(tc.tile_pool(name="wkp", bufs=ND))
    wvp = ctx.enter_context(tc.tile_pool(name="wvp", bufs=ND))
    wqp = ctx.enter_context(tc.tile_pool(name="wqp", bufs=ND))
    wop = ctx.enter_context(tc.tile_pool(name="wop", bufs=1))
    vts = ctx.enter_context(tc.tile_pool(name="vstage", bufs=2))
    # per-(tau,head) P slab: every pair's exp output lands contiguously so
    # the softmax denominator is one batched DVE tree per head; also means
    # pend depth is not limited by a small P-tile pool
    slabp = ctx.enter_context(tc.tile_pool(name="slabp", bufs=2))
    scrp = ctx.enter_context(tc.tile_pool(name="scrp", bufs=1))
    ptsums = ctx.enter_context(tc.tile_pool(name="ptsums", bufs=2))
    rcp = ctx.enter_context(tc.tile_pool(name="rcp", bufs=2))
    rcrp = ctx.enter_context(tc.tile_pool(name="rcrp", bufs=1))
    otp_pool = ctx.enter_context(tc.tile_pool(name="otsb", bufs=1))
    yb = ctx.enter_context(tc.tile_pool(name="ybounce", bufs=2))

    # ---- constants on the scalar queue (small, early)
    tri01 = consts.tile([128, 128], BF16, tag="tri01")
    nc.scalar.dma_start(tri01, maskTd)
    bqt = consts.tile([128, 4], F32, tag="bqt")
    nc.scalar.dma_start(bqt, bq.rearrange("(c p) -> p c", p=128))
    bkt = consts.tile([128, 1], F32, tag="bkt")
    nc.scalar.dma_start(bkt, bk.rearrange("(c p) -> p c", p=128))
    ident = consts.tile([128, 128], BF16, tag="ident")
    nc.scalar.dma_start(ident, identd)

    # ---- weights + x on the two fast queues (sync HWDGE, gpsimd SWDGE),
    # strictly in first-use order: wk, x(sg0), wv, wq, x(sg1..3), wo.
    xts = [xtp.tile([128, T], BF16, tag="xt", name=f"xt{d}") for d in range(ND)]
    wkts = [wkp.tile([128, DH], BF16, tag="wk", name=f"wk{d}") for d in range(ND)]
    wvts = [wvp.tile([128, DH], BF16, tag="wv", name=f"wv{d}") for d in range(ND)]
    wqts = [wqp.tile([128, QC], BF16, tag="wq", name=f"wq{d}") for d in range(ND)]
    wot = [wop.tile([128, D], BF16, tag=f"wo{c}", name=f"wo{c}") for c in range(HPG)]

    qlist = [nc.sync, nc.gpsimd]
    qi = 0

    def q_next():
        nonlocal qi
        eng = qlist[qi % 2]
        qi += 1
        return eng

    for d in range(ND):
        q_next().dma_start(wkts[d], wk[d * 128:(d + 1) * 128, :])
        q_next().dma_start(xts[d][:, 0:TW], xT[d * 128:(d + 1) * 128, 0:TW])
    for d in range(ND):
        q_next().dma_start(wvts[d], wv[d * 128:(d + 1) * 128, :])
    for d in range(ND):
        q_next().dma_start(wqts[d], wq[d * 128:(d + 1) * 128, :])
    for sg in range(1, NTAU):
        for d in range(ND):
            q_next().dma_start(
                xts[d][:, sg * TW:(sg + 1) * TW],
                xT[d * 128:(d + 1) * 128, sg * TW:(sg + 1) * TW])
    for c in range(HPG):
        q_next().dma_start(wot[c], wo[c * 128:(c + 1) * 128, :])

    # ---- HAM warm-up: real matmuls on a memset tile (no DMA dependency)
    # while the x DMAs land
    warm_in = consts.tile([128, 128], BF16, tag="warm_in")
    nc.vector.memset(warm_in, 0.0)

    def warm_fill(n):
        for w in range(n):
            wps = psot.tile([128, 128], F32, tag="ot", name="warm")
            nc.tensor.matmul(wps, warm_in, warm_in, start=True, stop=True)

    warm_fill(72)

    qt = [qkv.tile([128, T], BF16, tag=f"qt{j}", name=f"qt{j}") for j in range(HPG)]
    kt = qkv.tile([128, T], BF16, tag="kt")
    vv = qkv.tile([128, ND, 128], BF16, tag="vv")  # [s%128, s_block, dh]

    # ---- K projection chain for one sg column block (kt copy on DVE)
    def kchain(sg):
        ps = psacc.tile([128, TW], F32, tag="acc", name="psk")
        for d in range(ND):
            nc.tensor.matmul(ps, wkts[d], xts[d][:, sg * TW:(sg + 1) * TW],
                             start=(d == 0), stop=(d == ND - 1))
        nc.vector.tensor_scalar_add(kt[:, sg * TW:(sg + 1) * TW], ps,
                                    bkt[:, 0:1])

    # ---- V projection, split in two units: matmuls + vtt copy first, the
    # PE transposes several filler steps later so their LDWEIGHTS never
    # waits on the ACT queue while parking the PE FIFO
    vstash = {}

    def vmm(sg):
        ps2 = psacc.tile([128, TW], F32, tag="acc", name="psv")
        for d in range(ND):
            nc.tensor.matmul(ps2, wvts[d], xts[d][:, sg * TW:(sg + 1) * TW],
                             start=(d == 0), stop=(d == ND - 1))
        vtt = vts.tile([128, TW], BF16, tag="vt")
        nc.scalar.copy(vtt, ps2)
        vstash[sg] = vtt

    def vtr(sg):
        vtt = vstash.pop(sg)
        stg = psst.tile([128, TW], BF16, tag="st", name="vstg")
        for i in range(4):
            nc.tensor.transpose(stg[:, i * 128:(i + 1) * 128],
                                vtt[:, i * 128:(i + 1) * 128], ident)
        nc.vector.tensor_copy(vv[:, sg * 4:(sg + 1) * 4, :], stg)

    # ---- Q projection chain for one (t-tile, head block) (qt copy on DVE)
    def qchain(tau, cb):
        ps = psacc.tile([128, TW], F32, tag="acc", name="psq")
        for d in range(ND):
            nc.tensor.matmul(
                ps, wqts[d][:, cb * 128:(cb + 1) * 128],
                xts[d][:, tau * TW:(tau + 1) * TW],
                start=(d == 0), stop=(d == ND - 1))
        nc.vector.tensor_scalar_add(qt[cb][:, tau * TW:(tau + 1) * TW], ps,
                                    bqt[:, cb:cb + 1])

    # ---- output projection m-block, split emission: c0..2 accumulate,
    # then c3 + copy + store
    ots = [otp_pool.tile([128, T], BF16, tag=f"ot{j}", name=f"ots{j}")
           for j in range(HPG)]

    def oproj_start(tau, m, pool):
        if pool is psst:
            big = pool.tile([128, 2, TW], F32, tag="st", name="ypst")
            yp = big[:, 0, :]
        else:
            yp = pool.tile([128, TW], F32, tag="acc", name="yp")
        for c in range(HPG - 1):
            nc.tensor.matmul(
                yp, wot[c][:, m * 128:(m + 1) * 128],
                ots[c][:, tau * TW:(tau + 1) * TW],
                start=(c == 0), stop=False)
        return yp

    def oproj_fin(tau, m, yp, dve_copy=False):
        c = HPG - 1
        nc.tensor.matmul(
            yp, wot[c][:, m * 128:(m + 1) * 128],
            ots[c][:, tau * TW:(tau + 1) * TW],
            start=False, stop=True)
        ys = yb.tile([128, TW], BF16, tag="y", name="ys")
        if dve_copy:
            nc.vector.tensor_copy(ys, yp)
        else:
            nc.scalar.copy(ys, yp)
        # all stores on the sync HWDGE queue: gpsimd SWDGE triggers are
        # ~660ns each and serialize the drain tail
        nc.sync.dma_start(
            yT[m * 128:(m + 1) * 128, tau * TW:(tau + 1) * TW], ys)

    def oproj_block(tau, m):
        yp = oproj_start(tau, m, psacc)
        oproj_fin(tau, m, yp, dve_copy=(m % 2 == 1))

    # ---- softmax denominator: batched reduction tree over the head's P
    # slab (pair-slab granularity, flat 2D views for the DVE fast path),
    # emitted once per head at its last consume.
    def rowsum_tree(h):
        k = h["npair"]
        ptsum = h["ptsum"]
        PW = 2 * TW                        # one pair-slab = 1024 columns
        sl = h["slab"].rearrange("p a b c -> p (a b c)")
        scr4 = scrp.tile([128, MAXP - 1, 2, TW], BF16, tag="scr", name="scr")
        sc = scr4.rearrange("p a b c -> p (a b c)")
        add = nc.vector.tensor_add

        def seg(ap, a, b):
            return ap[:, a * PW:b * PW]

        half = k // 2
        add(seg(sc, 0, half), seg(sl, 0, half), seg(sl, half, 2 * half))
        if k == 2:        # npair by tau: 2, 4, 6, 8
            last = 0
        elif k == 4:
            add(seg(sc, 2, 3), seg(sc, 0, 1), seg(sc, 1, 2))
            last = 2
        elif k == 6:
            add(seg(sc, 3, 4), seg(sc, 0, 1), seg(sc, 1, 2))
            add(seg(sc, 4, 5), seg(sc, 3, 4), seg(sc, 2, 3))
            last = 4
        else:             # k == 8
            add(seg(sc, 4, 6), seg(sc, 0, 2), seg(sc, 2, 4))
            add(seg(sc, 6, 7), seg(sc, 4, 5), seg(sc, 5, 6))
            last = 6
        add(ptsum, sc[:, last * PW:last * PW + TW],
            sc[:, last * PW + TW:last * PW + PW])

    # ---- phase C: per-tau pipeline. flash(tau) spine = S -> exp -> PV with
    # the pend FIFO 3 pairs behind; fillers = proj chains of tau+1 +
    # budgeted oproj(tau-1) m-blocks, paced across the spine steps.

    pend = []
    norm_pend = []
    cc = [0]
    norms_done = [0] * NTAU

    def norm_flush(drain=False):
        while norm_pend and (drain or norm_pend[0][2] <= cc[0]):
            h, rc, _ = norm_pend.pop(0)
            rcr = rcrp.tile([128, TW], F32, tag="rcr")
            nc.vector.reciprocal_approx_fast(rcr, rc)
            nc.vector.tensor_mul(
                ots[h["j"]][:, h["tau"] * TW:(h["tau"] + 1) * TW],
                h["otp"], rcr)
            norms_done[h["tau"]] += 1

    def consume_one():
        cc[0] += 1
        norm_flush()
        h, q = pend.pop(0)
        otp, pairs, npair_ = h["otp"], h["pairs"], h["npair"]
        _, ptq, lo0, lo1 = pairs[q]
        first = h["ncons"] == 0
        h["ncons"] += 1
        last = h["ncons"] == npair_
        nc.tensor.matmul(otp[:, lo0:], vv[:, 2 * q, :],
                         ptq[:, 0, lo0:], start=first, stop=False)
        nc.tensor.matmul(otp[:, lo1:], vv[:, 2 * q + 1, :],
                         ptq[:, 1, lo1:], start=False, stop=last)
        del pairs[q]
        if last:
            rowsum_tree(h)
            rc = rcp.tile([128, TW], F32, tag="rc")
            nc.gpsimd.partition_all_reduce(rc, h["ptsum"], 128,
                                           bass_isa.ReduceOp.add)
            norm_pend.append([h, rc, cc[0] + 3])

    # prologue: projections for tau 0 run immediately
    kchain(0)
    vmm(0)
    for cb in range(HPG):
        qchain(0, cb)
    vtr(0)

    # filler units: ("proj", fn) always eligible; ("oproj", t, m) eligible
    # once norms_done[t] == HPG, capped per tau by the oproj budget.
    filler = []
    for tau in range(NTAU):
        units = []
        vtr_u = None
        if tau + 1 < NTAU:
            units.append(("proj", lambda sg=tau + 1: kchain(sg)))
            units.append(("proj", lambda sg=tau + 1: vmm(sg)))
            for cb in range(HPG):
                units.append(("proj", lambda sg=tau + 1, cb=cb: qchain(sg, cb)))
            vtr_u = ("proj", lambda sg=tau + 1: vtr(sg))
        units.extend(filler)
        if tau > 0:
            units.extend([("oproj", tau - 1, m) for m in range(ND)])
        if vtr_u is not None:
            # the transposes go several units after vmm so their LDWEIGHTS
            # never waits on the vtt copy queued behind exps
            units.insert(min(len(units), 10), vtr_u)
        filler = units

        nsb = 4 * tau + 4
        npair = nsb // 2
        total_steps = HPG * nsb
        step = 0
        fi = 0
        oproj_budget = [0, 99, 99, 99][tau]

        def try_fill():
            nonlocal fi, oproj_budget
            if fi >= len(filler):
                return
            u = filler[fi]
            if u[0] == "proj":
                u[1]()
                fi += 1
            elif oproj_budget > 0 and norms_done[u[1]] == HPG:
                oproj_block(u[1], u[2])
                fi += 1
                oproj_budget -= 1

        for j in range(HPG):
            slab_full = slabp.tile([128, MAXP, 2, TW], BF16, tag="slab",
                                   name="slab")
            h = {
                "otp": psot.tile([128, TW], F32, tag="ot", name="otp"),
                "ptsum": ptsums.tile([128, TW], BF16, tag="ps", name="ptsum"),
                "slab": slab_full,
                "pairs": {}, "tau": tau, "j": j, "npair": npair, "ncons": 0,
            }
            # last pair's exp only writes t >= 256; zero the stale strip the
            # rowsum tree would otherwise read (gpsimd: off the DVE path)
            nc.gpsimd.memset(slab_full[:, npair - 1, :, 0:256], 0.0)
            qslice = qt[j][:, tau * TW:(tau + 1) * TW]

            for sb in range(nsb):
                di = sb - 4 * tau
                lo = di * 128 if di >= 0 else 0   # valid t-range start
                if sb % 2 == 0:
                    stp = psst.tile([128, 2, TW], F32, tag="st")
                    ptq = slab_full[:, sb // 2]
                    h["pairs"][sb // 2] = [stp, ptq, lo, lo]
                pr = h["pairs"][sb // 2]
                pr[2 + sb % 2] = lo
                stp = pr[0]
                # plane 1 streams from the pair's lo0 so the batched pair
                # exp never touches bytes this tile didn't write; the
                # causally-invalid strip is masked out of pt after the exp
                slo = pr[2] if sb % 2 == 1 else lo
                nc.tensor.matmul(stp[:, sb % 2, slo:],
                                 kt[:, sb * 128:(sb + 1) * 128],
                                 qslice[:, slo:], start=True, stop=True)
                if sb % 2 == 1:
                    nc.scalar.activation(pr[1][:, :, pr[2]:],
                                         stp[:, :, pr[2]:], AF.Exp,
                                         scale=SCALE)
                    if di >= 0:
                        # causal triangle: zero pt's above-diagonal entries
                        # post-exp (keeps the DVE off the S->exp path);
                        # plane 1's [lo0, lo1) strip also zeroes (gpsimd)
                        # so the rowsum tree reads zeros there
                        nc.vector.tensor_mul(
                            pr[1][:, 0, pr[2]:pr[2] + 128],
                            pr[1][:, 0, pr[2]:pr[2] + 128], tri01)
                        nc.vector.tensor_mul(
                            pr[1][:, 1, pr[3]:pr[3] + 128],
                            pr[1][:, 1, pr[3]:pr[3] + 128], tri01)
                        if pr[3] > pr[2]:
                            nc.gpsimd.memset(pr[1][:, 1, pr[2]:pr[3]], 0.0)
                    pend.append([h, sb // 2])
                    if len(pend) > 5:
                        consume_one()
                step += 1
                while fi < len(filler) and fi < (step * len(filler)) // total_steps:
                    n0 = fi
                    try_fill()
                    if fi == n0:
                        break
        # boundary: flush remaining proj units (flash(tau+1) needs them);
        # oproj units carry over into the next tau's list
        rest = filler[fi:]
        filler = []
        for u in rest:
            if u[0] == "proj":
                u[1]()
            else:
                filler.append(u)

    # drain: consume remaining pairs, finish normalizes, leftover oproj
    while pend:
        consume_one()
    norm_flush(drain=True)
    for t, m in [(u[1], u[2]) for u in filler]:
        oproj_block(t, m)
    # final oproj in waves of 4 (2 acc + 2 st psum chains): c0..2 of the
    # wave execute before head 3's normalize mul lands; the c3s + copies
    # follow, alternating ACT/DVE so the tail drains on two engines.
    for w0, wn in ((0, 4), (4, 4), (8, 4), (12, 2), (14, 2)):
        wave = []
        for i, m in enumerate(range(w0, w0 + wn)):
            pool = psacc if i < 2 else psst
            wave.append((m, oproj_start(NTAU - 1, m, pool)))
        for m, yp in wave:
            oproj_fin(NTAU - 1, m, yp, dve_copy=(m % 2 == 1))


def _build_nc():
    if "nc" in _CACHE:
        return _CACHE["nc"]
    nc = bacc.Bacc("TRN2", target_bir_lowering=False, debug=False)
    xT = nc.dram_tensor("xT", [D, T], BF16, kind="ExternalInput").ap()
    wq = nc.dram_tensor("wq", [D, QC], BF16, kind="ExternalInput").ap()
    wk = nc.dram_tensor("wk", [D, DH], BF16, kind="ExternalInput").ap()
    wv = nc.dram_tensor("wv", [D, DH], BF16, kind="ExternalInput").ap()
    wo = nc.dram_tensor("wo", [QC, D], BF16, kind="ExternalInput").ap()
    bq = nc.dram_tensor("bq", [QC], F32, kind="ExternalInput").ap()
    bk = nc.dram_tensor("bk", [DH], F32, kind="ExternalInput").ap()
    maskTd = nc.dram_tensor("maskT", [128, 128], BF16, kind="ExternalInput").ap()
    identd = nc.dram_tensor("ident", [128, 128], BF16, kind="ExternalInput").ap()
    yT = nc.dram_tensor("yT", [D, T], BF16, kind="ExternalOutput").ap()

    with tile.TileContext(nc) as tc, ExitStack() as ctx:
        _body(ctx, tc, xT, wq, wk, wv, wo, bq, bk, maskTd, identd, yT)
    nc.compile()
    _CACHE["nc"] = nc
    return nc


def _host_consts():
    p = np.arange(128)[:, None]
    f = np.arange(128)[None, :]
    maskT = np.where(f >= p, 1.0, 0.0).astype(ml_dtypes.bfloat16)
    ident = np.eye(128, dtype=ml_dtypes.bfloat16)
    return maskT, ident


def make_in_maps(x, Wq, bq, Wk, bk, Wv, bv, Wo, bo):
    maskT, ident = _host_consts()
    bf = lambda a: np.ascontiguousarray(a).astype(ml_dtypes.bfloat16)

    xTb = [bf(x[b].T) for b in range(2)]
    in_maps = []
    for c in range(8):
        b, g = divmod(c, G)
        in_maps.append({
            "xT": xTb[b],
            "wq": bf(Wq[:, g * QC:(g + 1) * QC]),
            "wk": bf(Wk[:, g * DH:(g + 1) * DH]),
            "wv": bf(Wv[:, g * DH:(g + 1) * DH]),
            "wo": bf(Wo[g * QC:(g + 1) * QC, :]),
            "bq": np.ascontiguousarray(bq[g * QC:(g + 1) * QC]),
            "bk": np.ascontiguousarray(bk[g * DH:(g + 1) * DH]),
            "maskT": maskT,
            "ident": ident,
        })
    return in_maps


def kernel(x, Wq, bq, Wk, bk, Wv, bv, Wo, bo):
    global LAST_RESULTS
    x = np.asarray(x, np.float32)
    Wq = np.asarray(Wq, np.float32)
    Wk = np.asarray(Wk, np.float32)
    Wv = np.asarray(Wv, np.float32)
    Wo = np.asarray(Wo, np.float32)
    bq = np.asarray(bq, np.float32)
    bk = np.asarray(bk, np.float32)
    bv = np.asarray(bv, np.float32)
    bo = np.asarray(bo, np.float32)

    nc = _build_nc()
    in_maps = make_in_maps(x, Wq, bq, Wk, bk, Wv, bv, Wo, bo)

    res = run_bass_kernel_spmd(nc, in_maps, list(range(8)), trace=TRACE,
                               **TRACE_KW)
    LAST_RESULTS = res

    # V bias folded: bo_eff = bo + (bv per head) @ Wo
    bv_heads = np.repeat(bv.reshape(G, DH), HPG, axis=0).reshape(-1)
    bo_eff = bo + bv_heads @ Wo

    y = np.empty((2, T, D), np.float32)
    for b in range(2):
        acc = res.results[b * G + 0]["yT"].astype(np.float32)
        for g in range(1, G):
            acc += res.results[b * G + g]["yT"].astype(np.float32)
        y[b] = acc.T + bo_eff
    return y


# revision 52
# speedup vs baseline: 1.0396x; 1.0396x over previous
"""GroupedQueryAttention Trainium2 Bass kernel (v6).

Sharding: 8 cores = (B=2) x (G=4 KV groups). Each core computes, for its
(batch b, kv-group g): the 4 query heads' Q/K/V projections, causal flash
attention, and a partial output projection Y^T_g (bf16). Host sums the 4
partials per batch and adds an adjusted bias (bo + bv-term folded in).

Key structure (all transposed: token dim T on the free axis):
  xT[d, t]     uploaded pre-transposed from host (bf16)
  Q^T, K^T     from projection matmuls (W chunk stationary, xT moving)
  V^T -> V     PE transpose per 128-block, staged in the st PSUM slots
  S^T[s, t]  = (K^T s-block).T @ Q^T        (one 128-wide matmul per s-block)
  P^T        = exp(scale * S^T + mask)      (ACT, PSUM -> SBUF, bf16)
  O^T[dh, t] += (V s-block).T @ P^T         (PSUM accumulation over s-blocks)
  rowsum     += ones.T-style P sums (DVE bf16 adds per pair)
  Y^T[dm, t] = sum_c (Wo chunk).T @ O^T_c   (per 128-row dm block, bf16 out)

The V bias never enters the kernel: O = (P@(V0+1*bv))/rowsum = P@V0/rowsum
+ bv, and the constant bv contribution to Y is folded into bo on the host.

v6 scheduling (on the v2/v3 spine):
  - Unified filler stream: flash(tau) absorbs oproj(tau-1) m-blocks AND
    the K/V/Q projection chains of tau+1, so the tau boundary has no proj
    bubble. The V chain is split into two units - matmuls+copy early, PE
    transposes several steps later - so the transpose LDWEIGHTS never
    parks the PE FIFO on the ACT queue.
  - qt/kt PSUM->SBUF bias-copies on DVE (tensor_scalar_add), off the ACT
    FIFO that feeds exp.
  - pend FIFO depth 3 (PV consumes 3 pairs behind the S/exp front) to ride
    out exp latency spikes.
  - Light oproj budgets shift some m-blocks from the PE-bound flash(1/2)
    windows into the exp-bound flash(3); yT stores alternate the sync and
    gpsimd queues, and tail ys copies alternate ACT/DVE, so the drain ends
    within ~2us of the last matmul.

Normalize chain: ptsum adds (DVE, bf16) -> gpsimd 128-way all-reduce (f32)
-> reciprocal_approx_fast (DVE) -> mul (DVE), recip+mul deferred a few
consume slots so the DVE FIFO never blocks on the all-reduce.
"""

import sys

sys.path.insert(0, "/opt/trn_rl_repo")

from contextlib import ExitStack

import ml_dtypes
import numpy as np

import concourse.bass as bass  # noqa: F401
import concourse.tile as tile
from concourse import bacc, bass_isa, mybir
from concourse.bass_utils import run_bass_kernel_spmd

F32 = mybir.dt.float32
BF16 = mybir.dt.bfloat16
AF = mybir.ActivationFunctionType

D = 2048          # model dim
T = 2048          # tokens
DH = 128          # head dim
G = 4             # kv groups
HPG = 4           # query heads per group
QC = HPG * DH     # query cols per group = 512
ND = D // 128     # 16 contraction chunks
NTAU = 4          # t tiles of 512
TW = 512          # t tile width
SCALE = DH ** -0.5

TRACE = False
TRACE_KW = {}
LAST_RESULTS = None

_CACHE = {}


def _body(ctx, tc, xT, wq, wk, wv, wo, bq, bk, maskTd, identd, yT):
    nc = tc.nc

    # PSUM (16KB/partition exactly): acc 2x2KB + st-pair 2x4KB (shared with
    # V-transpose staging) + ot 2x2KB
    psacc = ctx.enter_context(tc.tile_pool(name="psacc", bufs=2, space="PSUM"))
    psst = ctx.enter_context(tc.tile_pool(name="psst", bufs=2, space="PSUM"))
    psot = ctx.enter_context(tc.tile_pool(name="psot", bufs=2, space="PSUM"))

    consts = ctx.enter_context(tc.tile_pool(name="consts", bufs=1))
    qkv = ctx.enter_context(tc.tile_pool(name="qkv", bufs=1))
    xtp = ctx.enter_context(tc.tile_pool(name="xtp", bufs=ND))
    wkp = ctx.enter_context(tc.tile_pool(name="wkp", bufs=ND))
    wvp = ctx.enter_context(tc.tile_pool(name="wvp", bufs=ND))
    wqp = ctx.enter_context(tc.tile_pool(name="wqp", bufs=ND))
    wop = ctx.enter_context(tc.tile_pool(name="wop", bufs=1))
    vts = ctx.enter_context(tc.tile_pool(name="vstage", bufs=2))
    # pend depth 6 keeps 6 unconsumed P tiles + 1 being written in flight
    ptp = ctx.enter_context(tc.tile_pool(name="ptp", bufs=7))
    ptsums = ctx.enter_context(tc.tile_pool(name="ptsums", bufs=2))
    nrm = ctx.enter_context(tc.tile_pool(name="norm", bufs=2))
    otp_pool = ctx.enter_context(tc.tile_pool(name="otsb", bufs=1))
    yb = ctx.enter_context(tc.tile_pool(name="ybounce", bufs=3))

    # ---- constants on the scalar queue (small, early)
    tri01 = consts.tile([128, 128], BF16, tag="tri01")
    nc.scalar.dma_start(tri01, maskTd)
    bqt = consts.tile([128, 4], F32, tag="bqt")
    nc.scalar.dma_start(bqt, bq.rearrange("(c p) -> p c", p=128))
    bkt = consts.tile([128, 1], F32, tag="bkt")
    nc.scalar.dma_start(bkt, bk.rearrange("(c p) -> p c", p=128))
    ident = consts.tile([128, 128], BF16, tag="ident")
    nc.scalar.dma_start(ident, identd)

    # ---- weights + x on the two fast queues (sync HWDGE, gpsimd SWDGE),
    # strictly in first-use order: wk, x(sg0), wv, wq, x(sg1..3), wo.
    xts = [xtp.tile([128, T], BF16, tag="xt", name=f"xt{d}") for d in range(ND)]
    wkts = [wkp.tile([128, DH], BF16, tag="wk", name=f"wk{d}") for d in range(ND)]
    wvts = [wvp.tile([128, DH], BF16, tag="wv", name=f"wv{d}") for d in range(ND)]
    wqts = [wqp.tile([128, QC], BF16, tag="wq", name=f"wq{d}") for d in range(ND)]
    wot = [wop.tile([128, D], BF16, tag=f"wo{c}", name=f"wo{c}") for c in range(HPG)]

    qlist = [nc.sync, nc.gpsimd]
    qi = 0

    def q_next():
        nonlocal qi
        eng = qlist[qi % 2]
        qi += 1
        return eng

    for d in range(ND):
        q_next().dma_start(wkts[d], wk[d * 128:(d + 1) * 128, :])
        q_next().dma_start(xts[d][:, 0:TW], xT[d * 128:(d + 1) * 128, 0:TW])
    for d in range(ND):
        q_next().dma_start(wvts[d], wv[d * 128:(d + 1) * 128, :])
    for d in range(ND):
        q_next().dma_start(wqts[d], wq[d * 128:(d + 1) * 128, :])
    for sg in range(1, NTAU):
        for d in range(ND):
            q_next().dma_start(
                xts[d][:, sg * TW:(sg + 1) * TW],
                xT[d * 128:(d + 1) * 128, sg * TW:(sg + 1) * TW])
    for c in range(HPG):
        q_next().dma_start(wot[c], wo[c * 128:(c + 1) * 128, :])

    # ---- HAM warm-up: real matmuls on a memset tile (no DMA dependency)
    # while the x DMAs land
    warm_in = consts.tile([128, 128], BF16, tag="warm_in")
    nc.vector.memset(warm_in, 0.0)

    def warm_fill(n):
        for w in range(n):
            wps = psot.tile([128, 128], F32, tag="ot", name="warm")
            nc.tensor.matmul(wps, warm_in, warm_in, start=True, stop=True)

    warm_fill(72)

    qt = [qkv.tile([128, T], BF16, tag=f"qt{j}", name=f"qt{j}") for j in range(HPG)]
    kt = qkv.tile([128, T], BF16, tag="kt")
    vv = qkv.tile([128, ND, 128], BF16, tag="vv")  # [s%128, s_block, dh]

    # ---- K projection chain for one sg column block (kt copy on DVE)
    def kchain(sg):
        ps = psacc.tile([128, TW], F32, tag="acc", name="psk")
        for d in range(ND):
            nc.tensor.matmul(ps, wkts[d], xts[d][:, sg * TW:(sg + 1) * TW],
                             start=(d == 0), stop=(d == ND - 1))
        nc.vector.tensor_scalar_add(kt[:, sg * TW:(sg + 1) * TW], ps,
                                    bkt[:, 0:1])

    # ---- V projection, split in two units: matmuls + vtt copy first, the
    # PE transposes several filler steps later so their LDWEIGHTS never
    # waits on the ACT queue while parking the PE FIFO
    vstash = {}

    def vmm(sg):
        ps2 = psacc.tile([128, TW], F32, tag="acc", name="psv")
        for d in range(ND):
            nc.tensor.matmul(ps2, wvts[d], xts[d][:, sg * TW:(sg + 1) * TW],
                             start=(d == 0), stop=(d == ND - 1))
        vtt = vts.tile([128, TW], BF16, tag="vt")
        nc.scalar.copy(vtt, ps2)
        vstash[sg] = vtt

    def vtr(sg):
        vtt = vstash.pop(sg)
        stg = psst.tile([128, TW], BF16, tag="st", name="vstg")
        for i in range(4):
            nc.tensor.transpose(stg[:, i * 128:(i + 1) * 128],
                                vtt[:, i * 128:(i + 1) * 128], ident)
        nc.vector.tensor_copy(vv[:, sg * 4:(sg + 1) * 4, :], stg)

    # ---- Q projection chain for one (t-tile, head block) (qt copy on DVE)
    def qchain(tau, cb):
        ps = psacc.tile([128, TW], F32, tag="acc", name="psq")
        for d in range(ND):
            nc.tensor.matmul(
                ps, wqts[d][:, cb * 128:(cb + 1) * 128],
                xts[d][:, tau * TW:(tau + 1) * TW],
                start=(d == 0), stop=(d == ND - 1))
        nc.vector.tensor_scalar_add(qt[cb][:, tau * TW:(tau + 1) * TW], ps,
                                    bqt[:, cb:cb + 1])

    # ---- output projection m-block, split emission: c0..2 accumulate,
    # then c3 + copy + store
    ots = [otp_pool.tile([128, T], BF16, tag=f"ot{j}", name=f"ots{j}")
           for j in range(HPG)]

    def oproj_start(tau, m, pool):
        if pool is psst:
            big = pool.tile([128, 2, TW], F32, tag="st", name="ypst")
            yp = big[:, 0, :]
        else:
            yp = pool.tile([128, TW], F32, tag="acc", name="yp")
        for c in range(HPG - 1):
            nc.tensor.matmul(
                yp, wot[c][:, m * 128:(m + 1) * 128],
                ots[c][:, tau * TW:(tau + 1) * TW],
                start=(c == 0), stop=False)
        return yp

    def oproj_fin(tau, m, yp, dve_copy=False):
        c = HPG - 1
        nc.tensor.matmul(
            yp, wot[c][:, m * 128:(m + 1) * 128],
            ots[c][:, tau * TW:(tau + 1) * TW],
            start=False, stop=True)
        ys = yb.tile([128, TW], BF16, tag="y", name="ys")
        if dve_copy:
            nc.vector.tensor_copy(ys, yp)
        else:
            nc.scalar.copy(ys, yp)
        # all stores on the sync HWDGE queue: gpsimd SWDGE triggers are
        # ~660ns each and serialize the drain tail
        nc.sync.dma_start(
            yT[m * 128:(m + 1) * 128, tau * TW:(tau + 1) * TW], ys)

    def oproj_block(tau, m):
        yp = oproj_start(tau, m, psacc)
        oproj_fin(tau, m, yp, dve_copy=(m % 2 == 1))

    # ---- phase C: per-tau pipeline. flash(tau) spine = S -> exp -> PV with
    # the pend FIFO 3 pairs behind; fillers = proj chains of tau+1 +
    # budgeted oproj(tau-1) m-blocks, paced across the spine steps.

    pend = []
    norm_pend = []
    cc = [0]
    norms_done = [0] * NTAU

    def norm_flush(drain=False):
        while norm_pend and (drain or norm_pend[0][2] <= cc[0]):
            h, rc, _ = norm_pend.pop(0)
            rcr = nrm.tile([128, TW], F32, tag="rcr")
            nc.vector.reciprocal_approx_fast(rcr, rc)
            nc.vector.tensor_mul(
                ots[h["j"]][:, h["tau"] * TW:(h["tau"] + 1) * TW],
                h["otp"], rcr)
            norms_done[h["tau"]] += 1

    def consume_one():
        cc[0] += 1
        norm_flush()
        h, q = pend.pop(0)
        otp, ptsum, pairs, npair_ = (
            h["otp"], h["ptsum"], h["pairs"], h["npair"])
        _, ptq, lo0, lo1 = pairs[q]
        first = h["ncons"] == 0
        h["ncons"] += 1
        last = h["ncons"] == npair_
        nc.tensor.matmul(otp[:, lo0:], vv[:, 2 * q, :],
                         ptq[:, 0, lo0:], start=first, stop=False)
        nc.tensor.matmul(otp[:, lo1:], vv[:, 2 * q + 1, :],
                         ptq[:, 1, lo1:], start=False, stop=last)
        # denominator: per-partition P-sums accumulate on DVE (bf16).
        # (gpsimd elementwise ops measure ~10x slower than DVE - keep off.)
        if first:
            nc.vector.tensor_copy(ptsum, ptq[:, 0, :])
        else:
            nc.vector.tensor_add(ptsum[:, lo0:], ptsum[:, lo0:],
                                 ptq[:, 0, lo0:])
        nc.vector.tensor_add(ptsum[:, lo1:], ptsum[:, lo1:], ptq[:, 1, lo1:])
        del pairs[q]
        if last:
            rc = nrm.tile([128, TW], F32, tag="rc")
            nc.gpsimd.partition_all_reduce(rc, ptsum, 128,
                                           bass_isa.ReduceOp.add)
            norm_pend.append([h, rc, cc[0] + 3])

    # prologue: projections for tau 0 run immediately
    kchain(0)
    vmm(0)
    for cb in range(HPG):
        qchain(0, cb)
    vtr(0)

    # filler units: ("proj", fn) always eligible; ("oproj", t, m) eligible
    # once norms_done[t] == HPG, capped per tau by the oproj budget.
    filler = []
    for tau in range(NTAU):
        units = []
        vtr_u = None
        if tau + 1 < NTAU:
            units.append(("proj", lambda sg=tau + 1: kchain(sg)))
            units.append(("proj", lambda sg=tau + 1: vmm(sg)))
            for cb in range(HPG):
                units.append(("proj", lambda sg=tau + 1, cb=cb: qchain(sg, cb)))
            vtr_u = ("proj", lambda sg=tau + 1: vtr(sg))
        units.extend(filler)
        if tau > 0:
            units.extend([("oproj", tau - 1, m) for m in range(ND)])
        if vtr_u is not None:
            # the transposes go several units after vmm so their LDWEIGHTS
            # never waits on the vtt copy queued behind exps
            units.insert(min(len(units), 10), vtr_u)
        filler = units

        nsb = 4 * tau + 4
        npair = nsb // 2
        total_steps = HPG * nsb
        step = 0
        fi = 0
        oproj_budget = [0, 99, 99, 99][tau]

        def try_fill():
            nonlocal fi, oproj_budget
            if fi >= len(filler):
                return
            u = filler[fi]
            if u[0] == "proj":
                u[1]()
                fi += 1
            elif oproj_budget > 0 and norms_done[u[1]] == HPG:
                oproj_block(u[1], u[2])
                fi += 1
                oproj_budget -= 1

        for j in range(HPG):
            h = {
                "otp": psot.tile([128, TW], F32, tag="ot", name="otp"),
                "ptsum": ptsums.tile([128, TW], BF16, tag="ps", name="ptsum"),
                "pairs": {}, "tau": tau, "j": j, "npair": npair, "ncons": 0,
            }
            qslice = qt[j][:, tau * TW:(tau + 1) * TW]

            for sb in range(nsb):
                di = sb - 4 * tau
                lo = di * 128 if di >= 0 else 0   # valid t-range start
                if sb % 2 == 0:
                    stp = psst.tile([128, 2, TW], F32, tag="st")
                    ptq = ptp.tile([128, 2, TW], BF16, tag="pt")
                    h["pairs"][sb // 2] = [stp, ptq, lo, lo]
                pr = h["pairs"][sb // 2]
                pr[2 + sb % 2] = lo
                stp = pr[0]
                # plane 1 streams from the pair's lo0 so the batched pair
                # exp never touches bytes this tile didn't write; the
                # causally-invalid strip is masked out of pt after the exp
                slo = pr[2] if sb % 2 == 1 else lo
                nc.tensor.matmul(stp[:, sb % 2, slo:],
                                 kt[:, sb * 128:(sb + 1) * 128],
                                 qslice[:, slo:], start=True, stop=True)
                if sb % 2 == 1:
                    nc.scalar.activation(pr[1][:, :, pr[2]:],
                                         stp[:, :, pr[2]:], AF.Exp,
                                         scale=SCALE)
                    if di >= 0:
                        # causal triangle: zero pt's above-diagonal entries
                        # post-exp (keeps the DVE off the S->exp path)
                        nc.vector.tensor_mul(
                            pr[1][:, 0, pr[2]:pr[2] + 128],
                            pr[1][:, 0, pr[2]:pr[2] + 128], tri01)
                        nc.vector.tensor_mul(
                            pr[1][:, 1, pr[3]:pr[3] + 128],
                            pr[1][:, 1, pr[3]:pr[3] + 128], tri01)
                    pend.append([h, sb // 2])
                    if len(pend) > 6:
                        consume_one()
                step += 1
                while fi < len(filler) and fi < (step * len(filler)) // total_steps:
                    n0 = fi
                    try_fill()
                    if fi == n0:
                        break
        # boundary: flush remaining proj units (flash(tau+1) needs them);
        # oproj units carry over into the next tau's list
        rest = filler[fi:]
        filler = []
        for u in rest:
            if u[0] == "proj":
                u[1]()
            else:
                filler.append(u)

    # drain: consume remaining pairs, finish normalizes, leftover oproj
    while pend:
        consume_one()
    norm_flush(drain=True)
    for t, m in [(u[1], u[2]) for u in filler]:
        oproj_block(t, m)
    # final oproj in waves of 4 (2 acc + 2 st psum chains): c0..2 of the
    # wave execute before head 3's normalize mul lands; the c3s + copies
    # follow, alternating ACT/DVE so the tail drains on two engines.
    for w0, wn in ((0, 4), (4, 4), (8, 4), (12, 2), (14, 2)):
        wave = []
        for i, m in enumerate(range(w0, w0 + wn)):
            pool = psacc if i < 2 else psst
            wave.append((m, oproj_start(NTAU - 1, m, pool)))
        for m, yp in wave:
            oproj_fin(NTAU - 1, m, yp, dve_copy=(m % 2 == 1))


def _build_nc():
    if "nc" in _CACHE:
        return _CACHE["nc"]
    nc = bacc.Bacc("TRN2", target_bir_lowering=False, debug=False)
    xT = nc.dram_tensor("xT", [D, T], BF16, kind="ExternalInput").ap()
    wq = nc.dram_tensor("wq", [D, QC], BF16, kind="ExternalInput").ap()
    wk = nc.dram_tensor("wk", [D, DH], BF16, kind="ExternalInput").ap()
    wv = nc.dram_tensor("wv", [D, DH], BF16, kind="ExternalInput").ap()
    wo = nc.dram_tensor("wo", [QC, D], BF16, kind="ExternalInput").ap()
    bq = nc.dram_tensor("bq", [QC], F32, kind="ExternalInput").ap()
    bk = nc.dram_tensor("bk", [DH], F32, kind="ExternalInput").ap()
    maskTd = nc.dram_tensor("maskT", [128, 128], BF16, kind="ExternalInput").ap()
    identd = nc.dram_tensor("ident", [128, 128], BF16, kind="ExternalInput").ap()
    yT = nc.dram_tensor("yT", [D, T], BF16, kind="ExternalOutput").ap()

    with tile.TileContext(nc) as tc, ExitStack() as ctx:
        _body(ctx, tc, xT, wq, wk, wv, wo, bq, bk, maskTd, identd, yT)
    nc.compile()
    _CACHE["nc"] = nc
    return nc


def _host_consts():
    p = np.arange(128)[:, None]
    f = np.arange(128)[None, :]
    maskT = np.where(f >= p, 1.0, 0.0).astype(ml_dtypes.bfloat16)
    ident = np.eye(128, dtype=ml_dtypes.bfloat16)
    return maskT, ident


def make_in_maps(x, Wq, bq, Wk, bk, Wv, bv, Wo, bo):
    maskT, ident = _host_consts()
    bf = lambda a: np.ascontiguousarray(a).astype(ml_dtypes.bfloat16)

    xTb = [bf(x[b].T) for b in range(2)]
    in_maps = []
    for c in range(8):
        b, g = divmod(c, G)
        in_maps.append({
            "xT": xTb[b],
            "wq": bf(Wq[:, g * QC:(g + 1) * QC]),
            "wk": bf(Wk[:, g * DH:(g + 1) * DH]),
            "wv": bf(Wv[:, g * DH:(g + 1) * DH]),
            "wo": bf(Wo[g * QC:(g + 1) * QC, :]),
            "bq": np.ascontiguousarray(bq[g * QC:(g + 1) * QC]),
            "bk": np.ascontiguousarray(bk[g * DH:(g + 1) * DH]),
            "maskT": maskT,
            "ident": ident,
        })
    return in_maps


def kernel(x, Wq, bq, Wk, bk, Wv, bv, Wo, bo):
    global LAST_RESULTS
    x = np.asarray(x, np.float32)
    Wq = np.asarray(Wq, np.float32)
    Wk = np.asarray(Wk, np.float32)
    Wv = np.asarray(Wv, np.float32)
    Wo = np.asarray(Wo, np.float32)
    bq = np.asarray(bq, np.float32)
    bk = np.asarray(bk, np.float32)
    bv = np.asarray(bv, np.float32)
    bo = np.asarray(bo, np.float32)

    nc = _build_nc()
    in_maps = make_in_maps(x, Wq, bq, Wk, bk, Wv, bv, Wo, bo)

    res = run_bass_kernel_spmd(nc, in_maps, list(range(8)), trace=TRACE,
                               **TRACE_KW)
    LAST_RESULTS = res

    # V bias folded: bo_eff = bo + (bv per head) @ Wo
    bv_heads = np.repeat(bv.reshape(G, DH), HPG, axis=0).reshape(-1)
    bo_eff = bo + bv_heads @ Wo

    y = np.empty((2, T, D), np.float32)
    for b in range(2):
        acc = res.results[b * G + 0]["yT"].astype(np.float32)
        for g in range(1, G):
            acc += res.results[b * G + g]["yT"].astype(np.float32)
        y[b] = acc.T + bo_eff
    return y


# revision 53
# speedup vs baseline: 1.0517x; 1.0116x over previous
"""GroupedQueryAttention Trainium2 Bass kernel (v6).

Sharding: 8 cores = (B=2) x (G=4 KV groups). Each core computes, for its
(batch b, kv-group g): the 4 query heads' Q/K/V projections, causal flash
attention, and a partial output projection Y^T_g (bf16). Host sums the 4
partials per batch and adds an adjusted bias (bo + bv-term folded in).

Key structure (all transposed: token dim T on the free axis):
  xT[d, t]     uploaded pre-transposed from host (bf16)
  Q^T, K^T     from projection matmuls (W chunk stationary, xT moving)
  V^T -> V     PE transpose per 128-block, staged in the st PSUM slots
  S^T[s, t]  = (K^T s-block).T @ Q^T        (one 128-wide matmul per s-block)
  P^T        = exp(scale * S^T + mask)      (ACT, PSUM -> SBUF, bf16)
  O^T[dh, t] += (V s-block).T @ P^T         (PSUM accumulation over s-blocks)
  rowsum     += ones.T-style P sums (DVE bf16 adds per pair)
  Y^T[dm, t] = sum_c (Wo chunk).T @ O^T_c   (per 128-row dm block, bf16 out)

The V bias never enters the kernel: O = (P@(V0+1*bv))/rowsum = P@V0/rowsum
+ bv, and the constant bv contribution to Y is folded into bo on the host.

v6 scheduling (on the v2/v3 spine):
  - Unified filler stream: flash(tau) absorbs oproj(tau-1) m-blocks AND
    the K/V/Q projection chains of tau+1, so the tau boundary has no proj
    bubble. The V chain is split into two units - matmuls+copy early, PE
    transposes several steps later - so the transpose LDWEIGHTS never
    parks the PE FIFO on the ACT queue.
  - qt/kt PSUM->SBUF bias-copies on DVE (tensor_scalar_add), off the ACT
    FIFO that feeds exp.
  - pend FIFO depth 3 (PV consumes 3 pairs behind the S/exp front) to ride
    out exp latency spikes.
  - Light oproj budgets shift some m-blocks from the PE-bound flash(1/2)
    windows into the exp-bound flash(3); yT stores alternate the sync and
    gpsimd queues, and tail ys copies alternate ACT/DVE, so the drain ends
    within ~2us of the last matmul.

Normalize chain: ptsum adds (DVE, bf16) -> gpsimd 128-way all-reduce (f32)
-> reciprocal_approx_fast (DVE) -> mul (DVE), recip+mul deferred a few
consume slots so the DVE FIFO never blocks on the all-reduce.
"""

import sys

sys.path.insert(0, "/opt/trn_rl_repo")

from contextlib import ExitStack

import ml_dtypes
import numpy as np

import concourse.bass as bass  # noqa: F401
import concourse.tile as tile
from concourse import bacc, bass_isa, mybir
from concourse.bass_utils import run_bass_kernel_spmd

F32 = mybir.dt.float32
BF16 = mybir.dt.bfloat16
AF = mybir.ActivationFunctionType

D = 2048          # model dim
T = 2048          # tokens
DH = 128          # head dim
G = 4             # kv groups
HPG = 4           # query heads per group
QC = HPG * DH     # query cols per group = 512
ND = D // 128     # 16 contraction chunks
NTAU = 4          # t tiles of 512
TW = 512          # t tile width
SCALE = DH ** -0.5

TRACE = False
TRACE_KW = {}
LAST_RESULTS = None

_CACHE = {}


def _body(ctx, tc, xT, wq, wk, wv, wo, bq, bk, maskTd, identd, yT):
    nc = tc.nc

    # PSUM (16KB/partition exactly): acc 2x2KB + st-pair 2x4KB (shared with
    # V-transpose staging) + ot 2x2KB
    psacc = ctx.enter_context(tc.tile_pool(name="psacc", bufs=2, space="PSUM"))
    psst = ctx.enter_context(tc.tile_pool(name="psst", bufs=2, space="PSUM"))
    psot = ctx.enter_context(tc.tile_pool(name="psot", bufs=2, space="PSUM"))

    consts = ctx.enter_context(tc.tile_pool(name="consts", bufs=1))
    qkv = ctx.enter_context(tc.tile_pool(name="qkv", bufs=1))
    xtp = ctx.enter_context(tc.tile_pool(name="xtp", bufs=ND))
    wkp = ctx.enter_context(tc.tile_pool(name="wkp", bufs=ND))
    wvp = ctx.enter_context(tc.tile_pool(name="wvp", bufs=ND))
    wqp = ctx.enter_context(tc.tile_pool(name="wqp", bufs=ND))
    wop = ctx.enter_context(tc.tile_pool(name="wop", bufs=1))
    vts = ctx.enter_context(tc.tile_pool(name="vstage", bufs=2))
    # pend depth 5 keeps 5 unconsumed P tiles + 1 being written in flight
    ptp = ctx.enter_context(tc.tile_pool(name="ptp", bufs=6))
    ptsums = ctx.enter_context(tc.tile_pool(name="ptsums", bufs=2))
    nrm = ctx.enter_context(tc.tile_pool(name="norm", bufs=2))
    otp_pool = ctx.enter_context(tc.tile_pool(name="otsb", bufs=1))
    yb = ctx.enter_context(tc.tile_pool(name="ybounce", bufs=3))

    # ---- constants on the scalar queue (small, early)
    tri01 = consts.tile([128, 128], BF16, tag="tri01")
    nc.scalar.dma_start(tri01, maskTd)
    bqt = consts.tile([128, 4], F32, tag="bqt")
    nc.scalar.dma_start(bqt, bq.rearrange("(c p) -> p c", p=128))
    bkt = consts.tile([128, 1], F32, tag="bkt")
    nc.scalar.dma_start(bkt, bk.rearrange("(c p) -> p c", p=128))
    ident = consts.tile([128, 128], BF16, tag="ident")
    nc.scalar.dma_start(ident, identd)

    # ---- weights + x on the two fast queues (sync HWDGE, gpsimd SWDGE),
    # strictly in first-use order: wk, x(sg0), wv, wq, x(sg1..3), wo.
    xts = [xtp.tile([128, T], BF16, tag="xt", name=f"xt{d}") for d in range(ND)]
    wkts = [wkp.tile([128, DH], BF16, tag="wk", name=f"wk{d}") for d in range(ND)]
    wvts = [wvp.tile([128, DH], BF16, tag="wv", name=f"wv{d}") for d in range(ND)]
    wqts = [wqp.tile([128, QC], BF16, tag="wq", name=f"wq{d}") for d in range(ND)]
    wot = [wop.tile([128, D], BF16, tag=f"wo{c}", name=f"wo{c}") for c in range(HPG)]

    qlist = [nc.sync, nc.gpsimd]
    qi = 0

    def q_next():
        nonlocal qi
        eng = qlist[qi % 2]
        qi += 1
        return eng

    for d in range(ND):
        q_next().dma_start(wkts[d], wk[d * 128:(d + 1) * 128, :])
        q_next().dma_start(xts[d][:, 0:TW], xT[d * 128:(d + 1) * 128, 0:TW])
    for d in range(ND):
        q_next().dma_start(wvts[d], wv[d * 128:(d + 1) * 128, :])
    for d in range(ND):
        q_next().dma_start(wqts[d], wq[d * 128:(d + 1) * 128, :])
    for sg in range(1, NTAU):
        for d in range(ND):
            q_next().dma_start(
                xts[d][:, sg * TW:(sg + 1) * TW],
                xT[d * 128:(d + 1) * 128, sg * TW:(sg + 1) * TW])
    for c in range(HPG):
        q_next().dma_start(wot[c], wo[c * 128:(c + 1) * 128, :])

    # ---- HAM warm-up: real matmuls on a memset tile (no DMA dependency)
    # while the x DMAs land
    warm_in = consts.tile([128, 128], BF16, tag="warm_in")
    nc.vector.memset(warm_in, 0.0)

    def warm_fill(n):
        for w in range(n):
            wps = psot.tile([128, 128], F32, tag="ot", name="warm")
            nc.tensor.matmul(wps, warm_in, warm_in, start=True, stop=True)

    warm_fill(72)

    qt = [qkv.tile([128, T], BF16, tag=f"qt{j}", name=f"qt{j}") for j in range(HPG)]
    kt = qkv.tile([128, T], BF16, tag="kt")
    vv = qkv.tile([128, ND, 128], BF16, tag="vv")  # [s%128, s_block, dh]

    # ---- K projection chain for one sg column block (kt copy on DVE)
    def kchain(sg):
        ps = psacc.tile([128, TW], F32, tag="acc", name="psk")
        for d in range(ND):
            nc.tensor.matmul(ps, wkts[d], xts[d][:, sg * TW:(sg + 1) * TW],
                             start=(d == 0), stop=(d == ND - 1))
        nc.vector.tensor_scalar_add(kt[:, sg * TW:(sg + 1) * TW], ps,
                                    bkt[:, 0:1])

    # ---- V projection, split in two units: matmuls + vtt copy first, the
    # PE transposes several filler steps later so their LDWEIGHTS never
    # waits on the ACT queue while parking the PE FIFO
    vstash = {}

    def vmm(sg):
        ps2 = psacc.tile([128, TW], F32, tag="acc", name="psv")
        for d in range(ND):
            nc.tensor.matmul(ps2, wvts[d], xts[d][:, sg * TW:(sg + 1) * TW],
                             start=(d == 0), stop=(d == ND - 1))
        vtt = vts.tile([128, TW], BF16, tag="vt")
        nc.scalar.copy(vtt, ps2)
        vstash[sg] = vtt

    def vtr(sg):
        vtt = vstash.pop(sg)
        stg = psst.tile([128, TW], BF16, tag="st", name="vstg")
        for i in range(4):
            nc.tensor.transpose(stg[:, i * 128:(i + 1) * 128],
                                vtt[:, i * 128:(i + 1) * 128], ident)
        nc.vector.tensor_copy(vv[:, sg * 4:(sg + 1) * 4, :], stg)

    # ---- Q projection chain for one (t-tile, head block) (qt copy on DVE)
    def qchain(tau, cb):
        ps = psacc.tile([128, TW], F32, tag="acc", name="psq")
        for d in range(ND):
            nc.tensor.matmul(
                ps, wqts[d][:, cb * 128:(cb + 1) * 128],
                xts[d][:, tau * TW:(tau + 1) * TW],
                start=(d == 0), stop=(d == ND - 1))
        nc.vector.tensor_scalar_add(qt[cb][:, tau * TW:(tau + 1) * TW], ps,
                                    bqt[:, cb:cb + 1])

    # ---- output projection m-block, split emission: c0..2 accumulate,
    # then c3 + copy + store
    ots = [otp_pool.tile([128, T], BF16, tag=f"ot{j}", name=f"ots{j}")
           for j in range(HPG)]

    def oproj_start(tau, m, pool):
        if pool is psst:
            big = pool.tile([128, 2, TW], F32, tag="st", name="ypst")
            yp = big[:, 0, :]
        else:
            yp = pool.tile([128, TW], F32, tag="acc", name="yp")
        for c in range(HPG - 1):
            nc.tensor.matmul(
                yp, wot[c][:, m * 128:(m + 1) * 128],
                ots[c][:, tau * TW:(tau + 1) * TW],
                start=(c == 0), stop=False)
        return yp

    def oproj_fin(tau, m, yp, dve_copy=False):
        c = HPG - 1
        nc.tensor.matmul(
            yp, wot[c][:, m * 128:(m + 1) * 128],
            ots[c][:, tau * TW:(tau + 1) * TW],
            start=False, stop=True)
        ys = yb.tile([128, TW], BF16, tag="y", name="ys")
        if dve_copy:
            nc.vector.tensor_copy(ys, yp)
        else:
            nc.scalar.copy(ys, yp)
        # all stores on the sync HWDGE queue: gpsimd SWDGE triggers are
        # ~660ns each and serialize the drain tail
        nc.sync.dma_start(
            yT[m * 128:(m + 1) * 128, tau * TW:(tau + 1) * TW], ys)

    def oproj_block(tau, m):
        yp = oproj_start(tau, m, psacc)
        oproj_fin(tau, m, yp, dve_copy=(m % 2 == 1))

    # ---- phase C: per-tau pipeline. flash(tau) spine = S -> exp -> PV with
    # the pend FIFO 3 pairs behind; fillers = proj chains of tau+1 +
    # budgeted oproj(tau-1) m-blocks, paced across the spine steps.

    pend = []
    norm_pend = []
    cc = [0]
    norms_done = [0] * NTAU

    def norm_flush(drain=False):
        while norm_pend and (drain or norm_pend[0][2] <= cc[0]):
            h, rc, _ = norm_pend.pop(0)
            rcr = nrm.tile([128, TW], F32, tag="rcr")
            nc.vector.reciprocal_approx_fast(rcr, rc)
            nc.vector.tensor_mul(
                ots[h["j"]][:, h["tau"] * TW:(h["tau"] + 1) * TW],
                h["otp"], rcr)
            norms_done[h["tau"]] += 1

    def consume_one():
        cc[0] += 1
        norm_flush()
        h, q = pend.pop(0)
        otp, ptsum, pairs, npair_ = (
            h["otp"], h["ptsum"], h["pairs"], h["npair"])
        _, ptq, lo0, lo1 = pairs[q]
        first = h["ncons"] == 0
        h["ncons"] += 1
        last = h["ncons"] == npair_
        nc.tensor.matmul(otp[:, lo0:], vv[:, 2 * q, :],
                         ptq[:, 0, lo0:], start=first, stop=False)
        nc.tensor.matmul(otp[:, lo1:], vv[:, 2 * q + 1, :],
                         ptq[:, 1, lo1:], start=False, stop=last)
        # denominator: per-partition P-sums accumulate on DVE (bf16).
        # (gpsimd elementwise ops measure ~10x slower than DVE - keep off.)
        if first:
            nc.vector.tensor_copy(ptsum, ptq[:, 0, :])
        else:
            nc.vector.tensor_add(ptsum[:, lo0:], ptsum[:, lo0:],
                                 ptq[:, 0, lo0:])
        nc.vector.tensor_add(ptsum[:, lo1:], ptsum[:, lo1:], ptq[:, 1, lo1:])
        del pairs[q]
        if last:
            rc = nrm.tile([128, TW], F32, tag="rc")
            nc.gpsimd.partition_all_reduce(rc, ptsum, 128,
                                           bass_isa.ReduceOp.add)
            norm_pend.append([h, rc, cc[0] + 3])

    # prologue: projections for tau 0 run immediately
    kchain(0)
    vmm(0)
    for cb in range(HPG):
        qchain(0, cb)
    vtr(0)

    # filler units: ("proj", fn) always eligible; ("oproj", t, m) eligible
    # once norms_done[t] == HPG, capped per tau by the oproj budget.
    filler = []
    for tau in range(NTAU):
        units = []
        vtr_u = None
        if tau + 1 < NTAU:
            units.append(("proj", lambda sg=tau + 1: kchain(sg)))
            units.append(("proj", lambda sg=tau + 1: vmm(sg)))
            for cb in range(HPG):
                units.append(("proj", lambda sg=tau + 1, cb=cb: qchain(sg, cb)))
            vtr_u = ("proj", lambda sg=tau + 1: vtr(sg))
        units.extend(filler)
        if tau > 0:
            units.extend([("oproj", tau - 1, m) for m in range(ND)])
        if vtr_u is not None:
            # the transposes go several units after vmm so their LDWEIGHTS
            # never waits on the vtt copy queued behind exps
            units.insert(min(len(units), 10), vtr_u)
        filler = units

        nsb = 4 * tau + 4
        npair = nsb // 2
        total_steps = HPG * nsb
        step = 0
        fi = 0
        oproj_budget = [0, 99, 99, 99][tau]

        def try_fill():
            nonlocal fi, oproj_budget
            if fi >= len(filler):
                return
            u = filler[fi]
            if u[0] == "proj":
                u[1]()
                fi += 1
            elif oproj_budget > 0 and norms_done[u[1]] == HPG:
                oproj_block(u[1], u[2])
                fi += 1
                oproj_budget -= 1

        for j in range(HPG):
            h = {
                "otp": psot.tile([128, TW], F32, tag="ot", name="otp"),
                "ptsum": ptsums.tile([128, TW], BF16, tag="ps", name="ptsum"),
                "pairs": {}, "tau": tau, "j": j, "npair": npair, "ncons": 0,
            }
            qslice = qt[j][:, tau * TW:(tau + 1) * TW]

            for sb in range(nsb):
                di = sb - 4 * tau
                lo = di * 128 if di >= 0 else 0   # valid t-range start
                if sb % 2 == 0:
                    stp = psst.tile([128, 2, TW], F32, tag="st")
                    ptq = ptp.tile([128, 2, TW], BF16, tag="pt")
                    h["pairs"][sb // 2] = [stp, ptq, lo, lo]
                pr = h["pairs"][sb // 2]
                pr[2 + sb % 2] = lo
                stp = pr[0]
                # plane 1 streams from the pair's lo0 so the batched pair
                # exp never touches bytes this tile didn't write; the
                # causally-invalid strip is masked out of pt after the exp
                slo = pr[2] if sb % 2 == 1 else lo
                nc.tensor.matmul(stp[:, sb % 2, slo:],
                                 kt[:, sb * 128:(sb + 1) * 128],
                                 qslice[:, slo:], start=True, stop=True)
                if sb % 2 == 1:
                    nc.scalar.activation(pr[1][:, :, pr[2]:],
                                         stp[:, :, pr[2]:], AF.Exp,
                                         scale=SCALE)
                    if di >= 0:
                        # causal triangle: zero pt's above-diagonal entries
                        # post-exp (keeps the DVE off the S->exp path)
                        nc.vector.tensor_mul(
                            pr[1][:, 0, pr[2]:pr[2] + 128],
                            pr[1][:, 0, pr[2]:pr[2] + 128], tri01)
                        nc.vector.tensor_mul(
                            pr[1][:, 1, pr[3]:pr[3] + 128],
                            pr[1][:, 1, pr[3]:pr[3] + 128], tri01)
                    pend.append([h, sb // 2])
                    if len(pend) > 5:
                        consume_one()
                step += 1
                while fi < len(filler) and fi < (step * len(filler)) // total_steps:
                    n0 = fi
                    try_fill()
                    if fi == n0:
                        break
        # boundary: flush remaining proj units (flash(tau+1) needs them);
        # oproj units carry over into the next tau's list
        rest = filler[fi:]
        filler = []
        for u in rest:
            if u[0] == "proj":
                u[1]()
            else:
                filler.append(u)

    # drain: consume remaining pairs, finish normalizes, leftover oproj
    while pend:
        consume_one()
    norm_flush(drain=True)
    for t, m in [(u[1], u[2]) for u in filler]:
        oproj_block(t, m)
    # final oproj in waves of 4 (2 acc + 2 st psum chains): c0..2 of the
    # wave execute before head 3's normalize mul lands; the c3s + copies
    # follow, alternating ACT/DVE so the tail drains on two engines.
    for w0, wn in ((0, 4), (4, 4), (8, 4), (12, 2), (14, 2)):
        wave = []
        for i, m in enumerate(range(w0, w0 + wn)):
            pool = psacc if i < 2 else psst
            wave.append((m, oproj_start(NTAU - 1, m, pool)))
        for m, yp in wave:
            oproj_fin(NTAU - 1, m, yp, dve_copy=(m % 2 == 1))


def _build_nc():
    if "nc" in _CACHE:
        return _CACHE["nc"]
    nc = bacc.Bacc("TRN2", target_bir_lowering=False, debug=False)
    xT = nc.dram_tensor("xT", [D, T], BF16, kind="ExternalInput").ap()
    wq = nc.dram_tensor("wq", [D, QC], BF16, kind="ExternalInput").ap()
    wk = nc.dram_tensor("wk", [D, DH], BF16, kind="ExternalInput").ap()
    wv = nc.dram_tensor("wv", [D, DH], BF16, kind="ExternalInput").ap()
    wo = nc.dram_tensor("wo", [QC, D], BF16, kind="ExternalInput").ap()
    bq = nc.dram_tensor("bq", [QC], F32, kind="ExternalInput").ap()
    bk = nc.dram_tensor("bk", [DH], F32, kind="ExternalInput").ap()
    maskTd = nc.dram_tensor("maskT", [128, 128], BF16, kind="ExternalInput").ap()
    identd = nc.dram_tensor("ident", [128, 128], BF16, kind="ExternalInput").ap()
    yT = nc.dram_tensor("yT", [D, T], BF16, kind="ExternalOutput").ap()

    with tile.TileContext(nc) as tc, ExitStack() as ctx:
        _body(ctx, tc, xT, wq, wk, wv, wo, bq, bk, maskTd, identd, yT)
    nc.compile()
    _CACHE["nc"] = nc
    return nc


def _host_consts():
    p = np.arange(128)[:, None]
    f = np.arange(128)[None, :]
    maskT = np.where(f >= p, 1.0, 0.0).astype(ml_dtypes.bfloat16)
    ident = np.eye(128, dtype=ml_dtypes.bfloat16)
    return maskT, ident


def make_in_maps(x, Wq, bq, Wk, bk, Wv, bv, Wo, bo):
    maskT, ident = _host_consts()
    bf = lambda a: np.ascontiguousarray(a).astype(ml_dtypes.bfloat16)

    xTb = [bf(x[b].T) for b in range(2)]
    in_maps = []
    for c in range(8):
        b, g = divmod(c, G)
        in_maps.append({
            "xT": xTb[b],
            "wq": bf(Wq[:, g * QC:(g + 1) * QC]),
            "wk": bf(Wk[:, g * DH:(g + 1) * DH]),
            "wv": bf(Wv[:, g * DH:(g + 1) * DH]),
            "wo": bf(Wo[g * QC:(g + 1) * QC, :]),
            "bq": np.ascontiguousarray(bq[g * QC:(g + 1) * QC]),
            "bk": np.ascontiguousarray(bk[g * DH:(g + 1) * DH]),
            "maskT": maskT,
            "ident": ident,
        })
    return in_maps


def kernel(x, Wq, bq, Wk, bk, Wv, bv, Wo, bo):
    global LAST_RESULTS
    x = np.asarray(x, np.float32)
    Wq = np.asarray(Wq, np.float32)
    Wk = np.asarray(Wk, np.float32)
    Wv = np.asarray(Wv, np.float32)
    Wo = np.asarray(Wo, np.float32)
    bq = np.asarray(bq, np.float32)
    bk = np.asarray(bk, np.float32)
    bv = np.asarray(bv, np.float32)
    bo = np.asarray(bo, np.float32)

    nc = _build_nc()
    in_maps = make_in_maps(x, Wq, bq, Wk, bk, Wv, bv, Wo, bo)

    res = run_bass_kernel_spmd(nc, in_maps, list(range(8)), trace=TRACE,
                               **TRACE_KW)
    LAST_RESULTS = res

    # V bias folded: bo_eff = bo + (bv per head) @ Wo
    bv_heads = np.repeat(bv.reshape(G, DH), HPG, axis=0).reshape(-1)
    bo_eff = bo + bv_heads @ Wo

    y = np.empty((2, T, D), np.float32)
    for b in range(2):
        acc = res.results[b * G + 0]["yT"].astype(np.float32)
        for g in range(1, G):
            acc += res.results[b * G + g]["yT"].astype(np.float32)
        y[b] = acc.T + bo_eff
    return y


# revision 61
# speedup vs baseline: 1.0603x; 1.0082x over previous
"""GroupedQueryAttention Trainium2 Bass kernel (v6).

Sharding: 8 cores = (B=2) x (G=4 KV groups). Each core computes, for its
(batch b, kv-group g): the 4 query heads' Q/K/V projections, causal flash
attention, and a partial output projection Y^T_g (bf16). Host sums the 4
partials per batch and adds an adjusted bias (bo + bv-term folded in).

Key structure (all transposed: token dim T on the free axis):
  xT[d, t]     uploaded pre-transposed from host (bf16)
  Q^T, K^T     from projection matmuls (W chunk stationary, xT moving)
  V^T -> V     PE transpose per 128-block, staged in the st PSUM slots
  S^T[s, t]  = (K^T s-block).T @ Q^T        (one 128-wide matmul per s-block)
  P^T        = exp(scale * S^T + mask)      (ACT, PSUM -> SBUF, bf16)
  O^T[dh, t] += (V s-block).T @ P^T         (PSUM accumulation over s-blocks)
  rowsum     += ones.T-style P sums (DVE bf16 adds per pair)
  Y^T[dm, t] = sum_c (Wo chunk).T @ O^T_c   (per 128-row dm block, bf16 out)

The V bias never enters the kernel: O = (P@(V0+1*bv))/rowsum = P@V0/rowsum
+ bv, and the constant bv contribution to Y is folded into bo on the host.

v6 scheduling (on the v2/v3 spine):
  - Unified filler stream: flash(tau) absorbs oproj(tau-1) m-blocks AND
    the K/V/Q projection chains of tau+1, so the tau boundary has no proj
    bubble. The V chain is split into two units - matmuls+copy early, PE
    transposes several steps later - so the transpose LDWEIGHTS never
    parks the PE FIFO on the ACT queue.
  - qt/kt PSUM->SBUF bias-copies on DVE (tensor_scalar_add), off the ACT
    FIFO that feeds exp.
  - pend FIFO depth 3 (PV consumes 3 pairs behind the S/exp front) to ride
    out exp latency spikes.
  - Light oproj budgets shift some m-blocks from the PE-bound flash(1/2)
    windows into the exp-bound flash(3); yT stores alternate the sync and
    gpsimd queues, and tail ys copies alternate ACT/DVE, so the drain ends
    within ~2us of the last matmul.

Normalize chain: ptsum adds (DVE, bf16) -> gpsimd 128-way all-reduce (f32)
-> reciprocal_approx_fast (DVE) -> mul (DVE), recip+mul deferred a few
consume slots so the DVE FIFO never blocks on the all-reduce.
"""

import sys

sys.path.insert(0, "/opt/trn_rl_repo")

from contextlib import ExitStack

import ml_dtypes
import numpy as np

import concourse.bass as bass  # noqa: F401
import concourse.tile as tile
from concourse import bacc, bass_isa, mybir
from concourse.bass_utils import run_bass_kernel_spmd

F32 = mybir.dt.float32
BF16 = mybir.dt.bfloat16
AF = mybir.ActivationFunctionType

D = 2048          # model dim
T = 2048          # tokens
DH = 128          # head dim
G = 4             # kv groups
HPG = 4           # query heads per group
QC = HPG * DH     # query cols per group = 512
ND = D // 128     # 16 contraction chunks
NTAU = 4          # t tiles of 512
TW = 512          # t tile width
SCALE = DH ** -0.5

TRACE = False
TRACE_KW = {}
LAST_RESULTS = None

_CACHE = {}


def _body(ctx, tc, xT, wq, wk, wv, wo, bq, bk, maskTd, maskT2d, identd, yT):
    nc = tc.nc

    # PSUM (16KB/partition exactly): acc 2x2KB + st-pair 2x4KB (shared with
    # V-transpose staging) + ot 2x2KB
    psacc = ctx.enter_context(tc.tile_pool(name="psacc", bufs=2, space="PSUM"))
    psst = ctx.enter_context(tc.tile_pool(name="psst", bufs=2, space="PSUM"))
    psot = ctx.enter_context(tc.tile_pool(name="psot", bufs=2, space="PSUM"))

    consts = ctx.enter_context(tc.tile_pool(name="consts", bufs=1))
    qkv = ctx.enter_context(tc.tile_pool(name="qkv", bufs=1))
    xtp = ctx.enter_context(tc.tile_pool(name="xtp", bufs=ND))
    wkp = ctx.enter_context(tc.tile_pool(name="wkp", bufs=ND))
    wvp = ctx.enter_context(tc.tile_pool(name="wvp", bufs=ND))
    wqp = ctx.enter_context(tc.tile_pool(name="wqp", bufs=ND))
    wop = ctx.enter_context(tc.tile_pool(name="wop", bufs=1))
    vts = ctx.enter_context(tc.tile_pool(name="vstage", bufs=2))
    # pend depth 5 keeps 5 unconsumed P tiles + 1 being written in flight
    ptp = ctx.enter_context(tc.tile_pool(name="ptp", bufs=6))
    ptsums = ctx.enter_context(tc.tile_pool(name="ptsums", bufs=2))
    nrm = ctx.enter_context(tc.tile_pool(name="norm", bufs=2))
    otp_pool = ctx.enter_context(tc.tile_pool(name="otsb", bufs=1))
    yb = ctx.enter_context(tc.tile_pool(name="ybounce", bufs=3))

    # ---- constants on the scalar queue (small, early)
    tri01 = consts.tile([128, 128], BF16, tag="tri01")
    nc.scalar.dma_start(tri01, maskTd)
    tri256 = consts.tile([128, 256], BF16, tag="tri256")
    nc.scalar.dma_start(tri256, maskT2d)
    bqt = consts.tile([128, 4], F32, tag="bqt")
    nc.scalar.dma_start(bqt, bq.rearrange("(c p) -> p c", p=128))
    bkt = consts.tile([128, 1], F32, tag="bkt")
    nc.scalar.dma_start(bkt, bk.rearrange("(c p) -> p c", p=128))
    ident = consts.tile([128, 128], BF16, tag="ident")
    nc.scalar.dma_start(ident, identd)

    # ---- weights + x on the two fast queues (sync HWDGE, gpsimd SWDGE),
    # strictly in first-use order: wk, x(sg0), wv, wq, x(sg1..3), wo.
    xts = [xtp.tile([128, T], BF16, tag="xt", name=f"xt{d}") for d in range(ND)]
    wkts = [wkp.tile([128, DH], BF16, tag="wk", name=f"wk{d}") for d in range(ND)]
    wvts = [wvp.tile([128, DH], BF16, tag="wv", name=f"wv{d}") for d in range(ND)]
    wqts = [wqp.tile([128, QC], BF16, tag="wq", name=f"wq{d}") for d in range(ND)]
    wot = [wop.tile([128, D], BF16, tag=f"wo{c}", name=f"wo{c}") for c in range(HPG)]

    qlist = [nc.sync, nc.gpsimd]
    qi = 0

    def q_next():
        nonlocal qi
        eng = qlist[qi % 2]
        qi += 1
        return eng

    for d in range(ND):
        q_next().dma_start(wkts[d], wk[d * 128:(d + 1) * 128, :])
        q_next().dma_start(xts[d][:, 0:TW], xT[d * 128:(d + 1) * 128, 0:TW])
    for d in range(ND):
        q_next().dma_start(wvts[d], wv[d * 128:(d + 1) * 128, :])
    for d in range(ND):
        q_next().dma_start(wqts[d], wq[d * 128:(d + 1) * 128, :])
    for sg in range(1, NTAU):
        for d in range(ND):
            q_next().dma_start(
                xts[d][:, sg * TW:(sg + 1) * TW],
                xT[d * 128:(d + 1) * 128, sg * TW:(sg + 1) * TW])
    for c in range(HPG):
        q_next().dma_start(wot[c], wo[c * 128:(c + 1) * 128, :])

    # ---- HAM warm-up: real matmuls on a memset tile (no DMA dependency)
    # while the x DMAs land
    warm_in = consts.tile([128, 128], BF16, tag="warm_in")
    nc.vector.memset(warm_in, 0.0)

    def warm_fill(n):
        for w in range(n):
            wps = psot.tile([128, 128], F32, tag="ot", name="warm")
            nc.tensor.matmul(wps, warm_in, warm_in, start=True, stop=True)

    warm_fill(72)

    qt = [qkv.tile([128, T], BF16, tag=f"qt{j}", name=f"qt{j}") for j in range(HPG)]
    kt = qkv.tile([128, T], BF16, tag="kt")
    vv = qkv.tile([128, ND, 128], BF16, tag="vv")  # [s%128, s_block, dh]

    # ---- K projection chain for one sg column block (kt copy on DVE)
    def kchain(sg):
        ps = psacc.tile([128, TW], F32, tag="acc", name="psk")
        for d in range(ND):
            nc.tensor.matmul(ps, wkts[d], xts[d][:, sg * TW:(sg + 1) * TW],
                             start=(d == 0), stop=(d == ND - 1))
        nc.vector.tensor_scalar_add(kt[:, sg * TW:(sg + 1) * TW], ps,
                                    bkt[:, 0:1])

    # ---- V projection, split in two units: matmuls + vtt copy first, the
    # PE transposes several filler steps later so their LDWEIGHTS never
    # waits on the ACT queue while parking the PE FIFO
    vstash = {}

    def vmm(sg):
        ps2 = psacc.tile([128, TW], F32, tag="acc", name="psv")
        for d in range(ND):
            nc.tensor.matmul(ps2, wvts[d], xts[d][:, sg * TW:(sg + 1) * TW],
                             start=(d == 0), stop=(d == ND - 1))
        vtt = vts.tile([128, TW], BF16, tag="vt")
        nc.scalar.copy(vtt, ps2)
        vstash[sg] = vtt

    def vtr(sg):
        vtt = vstash.pop(sg)
        stg = psst.tile([128, TW], BF16, tag="st", name="vstg")
        for i in range(4):
            nc.tensor.transpose(stg[:, i * 128:(i + 1) * 128],
                                vtt[:, i * 128:(i + 1) * 128], ident)
        nc.vector.tensor_copy(vv[:, sg * 4:(sg + 1) * 4, :], stg)

    # ---- Q projection chain for one (t-tile, head block) (qt copy on DVE)
    def qchain(tau, cb):
        ps = psacc.tile([128, TW], F32, tag="acc", name="psq")
        for d in range(ND):
            nc.tensor.matmul(
                ps, wqts[d][:, cb * 128:(cb + 1) * 128],
                xts[d][:, tau * TW:(tau + 1) * TW],
                start=(d == 0), stop=(d == ND - 1))
        nc.vector.tensor_scalar_add(qt[cb][:, tau * TW:(tau + 1) * TW], ps,
                                    bqt[:, cb:cb + 1])

    # ---- output projection m-block, split emission: c0..2 accumulate,
    # then c3 + copy + store
    ots = [otp_pool.tile([128, T], BF16, tag=f"ot{j}", name=f"ots{j}")
           for j in range(HPG)]

    def oproj_start(tau, m, pool):
        if pool is psst:
            big = pool.tile([128, 2, TW], F32, tag="st", name="ypst")
            yp = big[:, 0, :]
        else:
            yp = pool.tile([128, TW], F32, tag="acc", name="yp")
        for c in range(HPG - 1):
            nc.tensor.matmul(
                yp, wot[c][:, m * 128:(m + 1) * 128],
                ots[c][:, tau * TW:(tau + 1) * TW],
                start=(c == 0), stop=False)
        return yp

    def oproj_fin(tau, m, yp, dve_copy=False):
        c = HPG - 1
        nc.tensor.matmul(
            yp, wot[c][:, m * 128:(m + 1) * 128],
            ots[c][:, tau * TW:(tau + 1) * TW],
            start=False, stop=True)
        ys = yb.tile([128, TW], BF16, tag="y", name="ys")
        if dve_copy:
            nc.vector.tensor_copy(ys, yp)
        else:
            nc.scalar.copy(ys, yp)
        # all stores on the sync HWDGE queue: gpsimd SWDGE triggers are
        # ~660ns each and serialize the drain tail
        nc.sync.dma_start(
            yT[m * 128:(m + 1) * 128, tau * TW:(tau + 1) * TW], ys)

    def oproj_block(tau, m):
        yp = oproj_start(tau, m, psacc)
        oproj_fin(tau, m, yp, dve_copy=(m % 2 == 1))

    # ---- phase C: per-tau pipeline. flash(tau) spine = S -> exp -> PV with
    # the pend FIFO 3 pairs behind; fillers = proj chains of tau+1 +
    # budgeted oproj(tau-1) m-blocks, paced across the spine steps.

    pend = []
    norm_pend = []
    cc = [0]
    norms_done = [0] * NTAU

    def norm_flush(drain=False):
        while norm_pend and (drain or norm_pend[0][2] <= cc[0]):
            h, rc, _ = norm_pend.pop(0)
            rcr = nrm.tile([128, TW], F32, tag="rcr")
            nc.vector.reciprocal_approx_fast(rcr, rc)
            nc.vector.tensor_mul(
                ots[h["j"]][:, h["tau"] * TW:(h["tau"] + 1) * TW],
                h["otp"], rcr)
            norms_done[h["tau"]] += 1

    def consume_one():
        cc[0] += 1
        norm_flush()
        h, q = pend.pop(0)
        otp, ptsum2, pairs, npair_ = (
            h["otp"], h["ptsum2"], h["pairs"], h["npair"])
        _, ptq, lo0, lo1 = pairs[q]
        first = h["ncons"] == 0
        h["ncons"] += 1
        last = h["ncons"] == npair_
        nc.tensor.matmul(otp[:, lo0:], vv[:, 2 * q, :],
                         ptq[:, 0, lo0:], start=first, stop=False)
        nc.tensor.matmul(otp[:, lo1:], vv[:, 2 * q + 1, :],
                         ptq[:, 1, lo1:], start=False, stop=last)
        # denominator: both planes' per-partition P-sums accumulate in one
        # DVE op per pair (plane 1's [lo0, lo1) strip is mask-zeroed, so
        # the full-width add is garbage-free); folded at the head's end
        if first:
            nc.vector.tensor_copy(ptsum2, ptq)
        else:
            nc.vector.tensor_add(ptsum2[:, :, lo0:], ptsum2[:, :, lo0:],
                                 ptq[:, :, lo0:])
        del pairs[q]
        if last:
            rs = ptsums.tile([128, TW], BF16, tag="rsf", name="rsf")
            nc.vector.tensor_add(rs, ptsum2[:, 0, :], ptsum2[:, 1, :])
            rc = nrm.tile([128, TW], F32, tag="rc")
            nc.gpsimd.partition_all_reduce(rc, rs, 128,
                                           bass_isa.ReduceOp.add)
            norm_pend.append([h, rc, cc[0] + 3])

    # prologue: projections for tau 0 run immediately
    kchain(0)
    vmm(0)
    for cb in range(HPG):
        qchain(0, cb)
    vtr(0)

    # filler units: ("proj", fn) always eligible; ("oproj", t, m) eligible
    # once norms_done[t] == HPG, capped per tau by the oproj budget.
    filler = []
    for tau in range(NTAU):
        units = []
        vtr_u = None
        if tau + 1 < NTAU:
            units.append(("proj", lambda sg=tau + 1: kchain(sg)))
            units.append(("proj", lambda sg=tau + 1: vmm(sg)))
            for cb in range(HPG):
                units.append(("proj", lambda sg=tau + 1, cb=cb: qchain(sg, cb)))
            vtr_u = ("proj", lambda sg=tau + 1: vtr(sg))
        units.extend(filler)
        if tau > 0:
            units.extend([("oproj", tau - 1, m) for m in range(ND)])
        if vtr_u is not None:
            # the transposes go several units after vmm so their LDWEIGHTS
            # never waits on the vtt copy queued behind exps
            units.insert(min(len(units), 10), vtr_u)
        filler = units

        nsb = 4 * tau + 4
        npair = nsb // 2
        total_steps = HPG * nsb
        step = 0
        fi = 0
        oproj_budget = [0, 99, 99, 99][tau]

        def try_fill():
            nonlocal fi, oproj_budget
            if fi >= len(filler):
                return
            u = filler[fi]
            if u[0] == "proj":
                u[1]()
                fi += 1
            elif oproj_budget > 0 and norms_done[u[1]] == HPG:
                oproj_block(u[1], u[2])
                fi += 1
                oproj_budget -= 1

        for j in range(HPG):
            h = {
                "otp": psot.tile([128, TW], F32, tag="ot", name="otp"),
                "ptsum2": ptsums.tile([128, 2, TW], BF16, tag="ps",
                                      name="ptsum2"),
                "pairs": {}, "tau": tau, "j": j, "npair": npair, "ncons": 0,
            }
            qslice = qt[j][:, tau * TW:(tau + 1) * TW]

            for sb in range(nsb):
                di = sb - 4 * tau
                lo = di * 128 if di >= 0 else 0   # valid t-range start
                if sb % 2 == 0:
                    stp = psst.tile([128, 2, TW], F32, tag="st")
                    ptq = ptp.tile([128, 2, TW], BF16, tag="pt")
                    h["pairs"][sb // 2] = [stp, ptq, lo, lo]
                pr = h["pairs"][sb // 2]
                pr[2 + sb % 2] = lo
                stp = pr[0]
                # plane 1 streams from the pair's lo0 so the batched pair
                # exp never touches bytes this tile didn't write; the
                # causally-invalid strip is masked out of pt after the exp
                slo = pr[2] if sb % 2 == 1 else lo
                nc.tensor.matmul(stp[:, sb % 2, slo:],
                                 kt[:, sb * 128:(sb + 1) * 128],
                                 qslice[:, slo:], start=True, stop=True)
                if sb % 2 == 1:
                    nc.scalar.activation(pr[1][:, :, pr[2]:],
                                         stp[:, :, pr[2]:], AF.Exp,
                                         scale=SCALE)
                    if di >= 0:
                        # causal triangle: zero pt's above-diagonal entries
                        # post-exp (keeps the DVE off the S->exp path).
                        # plane 1's mask is 256 wide ([zeros | tri]) so its
                        # [lo0, lo1) garbage strip reads as zero downstream
                        nc.vector.tensor_mul(
                            pr[1][:, 0, pr[2]:pr[2] + 128],
                            pr[1][:, 0, pr[2]:pr[2] + 128], tri01)
                        nc.vector.tensor_mul(
                            pr[1][:, 1, pr[2]:pr[3] + 128],
                            pr[1][:, 1, pr[2]:pr[3] + 128], tri256)
                    pend.append([h, sb // 2])
                    if len(pend) > 5:
                        consume_one()
                step += 1
                while fi < len(filler) and fi < (step * len(filler)) // total_steps:
                    n0 = fi
                    try_fill()
                    if fi == n0:
                        break
        # boundary: flush remaining proj units (flash(tau+1) needs them);
        # oproj units carry over into the next tau's list
        rest = filler[fi:]
        filler = []
        for u in rest:
            if u[0] == "proj":
                u[1]()
            else:
                filler.append(u)

    # drain: consume remaining pairs, finish normalizes, leftover oproj
    while pend:
        consume_one()
    norm_flush(drain=True)
    for t, m in [(u[1], u[2]) for u in filler]:
        oproj_block(t, m)
    # final oproj in waves of 4 (2 acc + 2 st psum chains): c0..2 of the
    # wave execute before head 3's normalize mul lands; the c3s + copies
    # follow, alternating ACT/DVE so the tail drains on two engines.
    for w0, wn in ((0, 4), (4, 4), (8, 4), (12, 2), (14, 2)):
        wave = []
        for i, m in enumerate(range(w0, w0 + wn)):
            pool = psacc if i < 2 else psst
            wave.append((m, oproj_start(NTAU - 1, m, pool)))
        for m, yp in wave:
            oproj_fin(NTAU - 1, m, yp, dve_copy=(m % 2 == 1))


def _build_nc():
    if "nc" in _CACHE:
        return _CACHE["nc"]
    nc = bacc.Bacc("TRN2", target_bir_lowering=False, debug=False)
    xT = nc.dram_tensor("xT", [D, T], BF16, kind="ExternalInput").ap()
    wq = nc.dram_tensor("wq", [D, QC], BF16, kind="ExternalInput").ap()
    wk = nc.dram_tensor("wk", [D, DH], BF16, kind="ExternalInput").ap()
    wv = nc.dram_tensor("wv", [D, DH], BF16, kind="ExternalInput").ap()
    wo = nc.dram_tensor("wo", [QC, D], BF16, kind="ExternalInput").ap()
    bq = nc.dram_tensor("bq", [QC], F32, kind="ExternalInput").ap()
    bk = nc.dram_tensor("bk", [DH], F32, kind="ExternalInput").ap()
    maskTd = nc.dram_tensor("maskT", [128, 128], BF16, kind="ExternalInput").ap()
    maskT2d = nc.dram_tensor("maskT2", [128, 256], BF16, kind="ExternalInput").ap()
    identd = nc.dram_tensor("ident", [128, 128], BF16, kind="ExternalInput").ap()
    yT = nc.dram_tensor("yT", [D, T], BF16, kind="ExternalOutput").ap()

    with tile.TileContext(nc) as tc, ExitStack() as ctx:
        _body(ctx, tc, xT, wq, wk, wv, wo, bq, bk, maskTd, maskT2d, identd, yT)
    nc.compile()
    _CACHE["nc"] = nc
    return nc


def _host_consts():
    p = np.arange(128)[:, None]
    f = np.arange(128)[None, :]
    maskT = np.where(f >= p, 1.0, 0.0).astype(ml_dtypes.bfloat16)
    maskT2 = np.concatenate(
        [np.zeros((128, 128), ml_dtypes.bfloat16), maskT], axis=1)
    maskT2 = np.ascontiguousarray(maskT2)
    ident = np.eye(128, dtype=ml_dtypes.bfloat16)
    return maskT, maskT2, ident


def make_in_maps(x, Wq, bq, Wk, bk, Wv, bv, Wo, bo):
    maskT, maskT2, ident = _host_consts()
    bf = lambda a: np.ascontiguousarray(a).astype(ml_dtypes.bfloat16)

    xTb = [bf(x[b].T) for b in range(2)]
    in_maps = []
    for c in range(8):
        b, g = divmod(c, G)
        in_maps.append({
            "xT": xTb[b],
            "wq": bf(Wq[:, g * QC:(g + 1) * QC]),
            "wk": bf(Wk[:, g * DH:(g + 1) * DH]),
            "wv": bf(Wv[:, g * DH:(g + 1) * DH]),
            "wo": bf(Wo[g * QC:(g + 1) * QC, :]),
            "bq": np.ascontiguousarray(bq[g * QC:(g + 1) * QC]),
            "bk": np.ascontiguousarray(bk[g * DH:(g + 1) * DH]),
            "maskT": maskT,
            "maskT2": maskT2,
            "ident": ident,
        })
    return in_maps


def kernel(x, Wq, bq, Wk, bk, Wv, bv, Wo, bo):
    global LAST_RESULTS
    x = np.asarray(x, np.float32)
    Wq = np.asarray(Wq, np.float32)
    Wk = np.asarray(Wk, np.float32)
    Wv = np.asarray(Wv, np.float32)
    Wo = np.asarray(Wo, np.float32)
    bq = np.asarray(bq, np.float32)
    bk = np.asarray(bk, np.float32)
    bv = np.asarray(bv, np.float32)
    bo = np.asarray(bo, np.float32)

    nc = _build_nc()
    in_maps = make_in_maps(x, Wq, bq, Wk, bk, Wv, bv, Wo, bo)

    res = run_bass_kernel_spmd(nc, in_maps, list(range(8)), trace=TRACE,
                               **TRACE_KW)
    LAST_RESULTS = res

    # V bias folded: bo_eff = bo + (bv per head) @ Wo
    bv_heads = np.repeat(bv.reshape(G, DH), HPG, axis=0).reshape(-1)
    bo_eff = bo + bv_heads @ Wo

    y = np.empty((2, T, D), np.float32)
    for b in range(2):
        acc = res.results[b * G + 0]["yT"].astype(np.float32)
        for g in range(1, G):
            acc += res.results[b * G + g]["yT"].astype(np.float32)
        y[b] = acc.T + bo_eff
    return y


# revision 62
# speedup vs baseline: 1.0627x; 1.0023x over previous
"""GroupedQueryAttention Trainium2 Bass kernel (v6).

Sharding: 8 cores = (B=2) x (G=4 KV groups). Each core computes, for its
(batch b, kv-group g): the 4 query heads' Q/K/V projections, causal flash
attention, and a partial output projection Y^T_g (bf16). Host sums the 4
partials per batch and adds an adjusted bias (bo + bv-term folded in).

Key structure (all transposed: token dim T on the free axis):
  xT[d, t]     uploaded pre-transposed from host (bf16)
  Q^T, K^T     from projection matmuls (W chunk stationary, xT moving)
  V^T -> V     PE transpose per 128-block, staged in the st PSUM slots
  S^T[s, t]  = (K^T s-block).T @ Q^T        (one 128-wide matmul per s-block)
  P^T        = exp(scale * S^T + mask)      (ACT, PSUM -> SBUF, bf16)
  O^T[dh, t] += (V s-block).T @ P^T         (PSUM accumulation over s-blocks)
  rowsum     += ones.T-style P sums (DVE bf16 adds per pair)
  Y^T[dm, t] = sum_c (Wo chunk).T @ O^T_c   (per 128-row dm block, bf16 out)

The V bias never enters the kernel: O = (P@(V0+1*bv))/rowsum = P@V0/rowsum
+ bv, and the constant bv contribution to Y is folded into bo on the host.

v6 scheduling (on the v2/v3 spine):
  - Unified filler stream: flash(tau) absorbs oproj(tau-1) m-blocks AND
    the K/V/Q projection chains of tau+1, so the tau boundary has no proj
    bubble. The V chain is split into two units - matmuls+copy early, PE
    transposes several steps later - so the transpose LDWEIGHTS never
    parks the PE FIFO on the ACT queue.
  - qt/kt PSUM->SBUF bias-copies on DVE (tensor_scalar_add), off the ACT
    FIFO that feeds exp.
  - pend FIFO depth 3 (PV consumes 3 pairs behind the S/exp front) to ride
    out exp latency spikes.
  - Light oproj budgets shift some m-blocks from the PE-bound flash(1/2)
    windows into the exp-bound flash(3); yT stores alternate the sync and
    gpsimd queues, and tail ys copies alternate ACT/DVE, so the drain ends
    within ~2us of the last matmul.

Normalize chain: ptsum adds (DVE, bf16) -> gpsimd 128-way all-reduce (f32)
-> reciprocal_approx_fast (DVE) -> mul (DVE), recip+mul deferred a few
consume slots so the DVE FIFO never blocks on the all-reduce.
"""

import sys

sys.path.insert(0, "/opt/trn_rl_repo")

from contextlib import ExitStack

import ml_dtypes
import numpy as np

import concourse.bass as bass  # noqa: F401
import concourse.tile as tile
from concourse import bacc, bass_isa, mybir
from concourse.bass_utils import run_bass_kernel_spmd

F32 = mybir.dt.float32
BF16 = mybir.dt.bfloat16
AF = mybir.ActivationFunctionType

D = 2048          # model dim
T = 2048          # tokens
DH = 128          # head dim
G = 4             # kv groups
HPG = 4           # query heads per group
QC = HPG * DH     # query cols per group = 512
ND = D // 128     # 16 contraction chunks
NTAU = 4          # t tiles of 512
TW = 512          # t tile width
SCALE = DH ** -0.5

TRACE = False
TRACE_KW = {}
LAST_RESULTS = None

_CACHE = {}


def _body(ctx, tc, xT, wq, wk, wv, wo, bq, bk, maskTd, maskT2d, identd, yT):
    nc = tc.nc

    # PSUM (16KB/partition exactly): acc 2x2KB + st-pair 2x4KB (shared with
    # V-transpose staging) + ot 2x2KB
    psacc = ctx.enter_context(tc.tile_pool(name="psacc", bufs=2, space="PSUM"))
    psst = ctx.enter_context(tc.tile_pool(name="psst", bufs=2, space="PSUM"))
    psot = ctx.enter_context(tc.tile_pool(name="psot", bufs=2, space="PSUM"))

    consts = ctx.enter_context(tc.tile_pool(name="consts", bufs=1))
    qkv = ctx.enter_context(tc.tile_pool(name="qkv", bufs=1))
    xtp = ctx.enter_context(tc.tile_pool(name="xtp", bufs=ND))
    wkp = ctx.enter_context(tc.tile_pool(name="wkp", bufs=ND))
    wvp = ctx.enter_context(tc.tile_pool(name="wvp", bufs=ND))
    wqp = ctx.enter_context(tc.tile_pool(name="wqp", bufs=ND))
    wop = ctx.enter_context(tc.tile_pool(name="wop", bufs=1))
    vts = ctx.enter_context(tc.tile_pool(name="vstage", bufs=2))
    # pend depth 5 keeps 5 unconsumed P tiles + 1 being written in flight
    ptp = ctx.enter_context(tc.tile_pool(name="ptp", bufs=6))
    ptsums = ctx.enter_context(tc.tile_pool(name="ptsums", bufs=2))
    nrm = ctx.enter_context(tc.tile_pool(name="norm", bufs=2))
    otp_pool = ctx.enter_context(tc.tile_pool(name="otsb", bufs=1))
    yb = ctx.enter_context(tc.tile_pool(name="ybounce", bufs=3))

    # ---- constants on the scalar queue (small, early)
    tri01 = consts.tile([128, 128], BF16, tag="tri01")
    nc.scalar.dma_start(tri01, maskTd)
    tri256 = consts.tile([128, 256], BF16, tag="tri256")
    nc.scalar.dma_start(tri256, maskT2d)
    bqt = consts.tile([128, 4], F32, tag="bqt")
    nc.scalar.dma_start(bqt, bq.rearrange("(c p) -> p c", p=128))
    bkt = consts.tile([128, 1], F32, tag="bkt")
    nc.scalar.dma_start(bkt, bk.rearrange("(c p) -> p c", p=128))
    ident = consts.tile([128, 128], BF16, tag="ident")
    nc.scalar.dma_start(ident, identd)

    # ---- weights + x on the two fast queues (sync HWDGE, gpsimd SWDGE),
    # strictly in first-use order: wk, x(sg0), wv, wq, x(sg1..3), wo.
    xts = [xtp.tile([128, T], BF16, tag="xt", name=f"xt{d}") for d in range(ND)]
    wkts = [wkp.tile([128, DH], BF16, tag="wk", name=f"wk{d}") for d in range(ND)]
    wvts = [wvp.tile([128, DH], BF16, tag="wv", name=f"wv{d}") for d in range(ND)]
    wqts = [wqp.tile([128, QC], BF16, tag="wq", name=f"wq{d}") for d in range(ND)]
    wot = [wop.tile([128, D], BF16, tag=f"wo{c}", name=f"wo{c}") for c in range(HPG)]

    qlist = [nc.sync, nc.gpsimd]
    qi = 0

    def q_next():
        nonlocal qi
        eng = qlist[qi % 2]
        qi += 1
        return eng

    for d in range(ND):
        q_next().dma_start(wkts[d], wk[d * 128:(d + 1) * 128, :])
        q_next().dma_start(xts[d][:, 0:TW], xT[d * 128:(d + 1) * 128, 0:TW])
    for d in range(ND):
        q_next().dma_start(wvts[d], wv[d * 128:(d + 1) * 128, :])
    for d in range(ND):
        q_next().dma_start(wqts[d], wq[d * 128:(d + 1) * 128, :])
    for sg in range(1, NTAU):
        for d in range(ND):
            q_next().dma_start(
                xts[d][:, sg * TW:(sg + 1) * TW],
                xT[d * 128:(d + 1) * 128, sg * TW:(sg + 1) * TW])
    for c in range(HPG):
        q_next().dma_start(wot[c], wo[c * 128:(c + 1) * 128, :])

    # ---- HAM warm-up: real matmuls on a memset tile (no DMA dependency)
    # while the x DMAs land
    warm_in = consts.tile([128, 128], BF16, tag="warm_in")
    nc.vector.memset(warm_in, 0.0)

    def warm_fill(n):
        for w in range(n):
            wps = psot.tile([128, 128], F32, tag="ot", name="warm")
            nc.tensor.matmul(wps, warm_in, warm_in, start=True, stop=True)

    warm_fill(72)

    qt = [qkv.tile([128, T], BF16, tag=f"qt{j}", name=f"qt{j}") for j in range(HPG)]
    kt = qkv.tile([128, T], BF16, tag="kt")
    vv = qkv.tile([128, ND, 128], BF16, tag="vv")  # [s%128, s_block, dh]

    # ---- K projection chain for one sg column block (kt copy on DVE)
    def kchain(sg):
        ps = psacc.tile([128, TW], F32, tag="acc", name="psk")
        for d in range(ND):
            nc.tensor.matmul(ps, wkts[d], xts[d][:, sg * TW:(sg + 1) * TW],
                             start=(d == 0), stop=(d == ND - 1))
        nc.vector.tensor_scalar_add(kt[:, sg * TW:(sg + 1) * TW], ps,
                                    bkt[:, 0:1])

    # ---- V projection, split in two units: matmuls + vtt copy first, the
    # PE transposes several filler steps later so their LDWEIGHTS never
    # waits on the ACT queue while parking the PE FIFO
    vstash = {}

    def vmm(sg):
        ps2 = psacc.tile([128, TW], F32, tag="acc", name="psv")
        for d in range(ND):
            nc.tensor.matmul(ps2, wvts[d], xts[d][:, sg * TW:(sg + 1) * TW],
                             start=(d == 0), stop=(d == ND - 1))
        vtt = vts.tile([128, TW], BF16, tag="vt")
        nc.scalar.copy(vtt, ps2)
        vstash[sg] = vtt

    def vtr(sg):
        vtt = vstash.pop(sg)
        stg = psst.tile([128, TW], BF16, tag="st", name="vstg")
        for i in range(4):
            nc.tensor.transpose(stg[:, i * 128:(i + 1) * 128],
                                vtt[:, i * 128:(i + 1) * 128], ident)
        nc.vector.tensor_copy(vv[:, sg * 4:(sg + 1) * 4, :], stg)

    # ---- Q projection chain for one (t-tile, head block) (qt copy on DVE)
    def qchain(tau, cb):
        ps = psacc.tile([128, TW], F32, tag="acc", name="psq")
        for d in range(ND):
            nc.tensor.matmul(
                ps, wqts[d][:, cb * 128:(cb + 1) * 128],
                xts[d][:, tau * TW:(tau + 1) * TW],
                start=(d == 0), stop=(d == ND - 1))
        nc.vector.tensor_scalar_add(qt[cb][:, tau * TW:(tau + 1) * TW], ps,
                                    bqt[:, cb:cb + 1])

    # ---- output projection m-block, split emission: c0..2 accumulate,
    # then c3 + copy + store
    ots = [otp_pool.tile([128, T], BF16, tag=f"ot{j}", name=f"ots{j}")
           for j in range(HPG)]

    def oproj_start(tau, m, pool):
        if pool is psst:
            big = pool.tile([128, 2, TW], F32, tag="st", name="ypst")
            yp = big[:, 0, :]
        else:
            yp = pool.tile([128, TW], F32, tag="acc", name="yp")
        for c in range(HPG - 1):
            nc.tensor.matmul(
                yp, wot[c][:, m * 128:(m + 1) * 128],
                ots[c][:, tau * TW:(tau + 1) * TW],
                start=(c == 0), stop=False)
        return yp

    def oproj_fin(tau, m, yp, dve_copy=False):
        c = HPG - 1
        nc.tensor.matmul(
            yp, wot[c][:, m * 128:(m + 1) * 128],
            ots[c][:, tau * TW:(tau + 1) * TW],
            start=False, stop=True)
        ys = yb.tile([128, TW], BF16, tag="y", name="ys")
        if dve_copy:
            nc.vector.tensor_copy(ys, yp)
        else:
            nc.scalar.copy(ys, yp)
        # all stores on the sync HWDGE queue: gpsimd SWDGE triggers are
        # ~660ns each and serialize the drain tail
        nc.sync.dma_start(
            yT[m * 128:(m + 1) * 128, tau * TW:(tau + 1) * TW], ys)

    def oproj_block(tau, m):
        yp = oproj_start(tau, m, psacc)
        oproj_fin(tau, m, yp, dve_copy=(m % 2 == 1))

    # ---- phase C: per-tau pipeline. flash(tau) spine = S -> exp -> PV with
    # the pend FIFO 3 pairs behind; fillers = proj chains of tau+1 +
    # budgeted oproj(tau-1) m-blocks, paced across the spine steps.

    pend = []
    norm_pend = []
    cc = [0]
    norms_done = [0] * NTAU

    def norm_flush(drain=False):
        while norm_pend and (drain or norm_pend[0][2] <= cc[0]):
            h, rc, _ = norm_pend.pop(0)
            rcr = nrm.tile([128, TW], F32, tag="rcr")
            nc.vector.reciprocal_approx_fast(rcr, rc)
            nc.vector.tensor_mul(
                ots[h["j"]][:, h["tau"] * TW:(h["tau"] + 1) * TW],
                h["otp"], rcr)
            norms_done[h["tau"]] += 1

    def consume_one():
        cc[0] += 1
        norm_flush()
        h, q = pend.pop(0)
        otp, ptsum2, pairs, npair_ = (
            h["otp"], h["ptsum2"], h["pairs"], h["npair"])
        _, ptq, lo0, lo1 = pairs[q]
        first = h["ncons"] == 0
        h["ncons"] += 1
        last = h["ncons"] == npair_
        nc.tensor.matmul(otp[:, lo0:], vv[:, 2 * q, :],
                         ptq[:, 0, lo0:], start=first, stop=False)
        nc.tensor.matmul(otp[:, lo1:], vv[:, 2 * q + 1, :],
                         ptq[:, 1, lo1:], start=False, stop=last)
        # denominator: both planes' per-partition P-sums accumulate in one
        # DVE op per pair (plane 1's [lo0, lo1) strip is mask-zeroed, so
        # the full-width add is garbage-free); folded at the head's end
        if first:
            nc.vector.tensor_copy(ptsum2, ptq)
        else:
            nc.vector.tensor_add(ptsum2[:, :, lo0:], ptsum2[:, :, lo0:],
                                 ptq[:, :, lo0:])
        del pairs[q]
        if last:
            rs = ptsums.tile([128, TW], BF16, tag="rsf", name="rsf")
            nc.vector.tensor_add(rs, ptsum2[:, 0, :], ptsum2[:, 1, :])
            rc = nrm.tile([128, TW], F32, tag="rc")
            nc.gpsimd.partition_all_reduce(rc, rs, 128,
                                           bass_isa.ReduceOp.add)
            # defer the DVE recip+mul 4 consume slots (~3.6us): the gpsimd
            # all-reduce takes ~3.5us and an early recip would park the DVE
            # FIFO (and the masks behind it) on that wait
            norm_pend.append([h, rc, cc[0] + 4])

    # prologue: projections for tau 0 run immediately
    kchain(0)
    vmm(0)
    for cb in range(HPG):
        qchain(0, cb)
    vtr(0)

    # filler units: ("proj", fn) always eligible; ("oproj", t, m) eligible
    # once norms_done[t] == HPG, capped per tau by the oproj budget.
    filler = []
    for tau in range(NTAU):
        units = []
        vtr_u = None
        if tau + 1 < NTAU:
            units.append(("proj", lambda sg=tau + 1: kchain(sg)))
            units.append(("proj", lambda sg=tau + 1: vmm(sg)))
            for cb in range(HPG):
                units.append(("proj", lambda sg=tau + 1, cb=cb: qchain(sg, cb)))
            vtr_u = ("proj", lambda sg=tau + 1: vtr(sg))
        units.extend(filler)
        if tau > 0:
            units.extend([("oproj", tau - 1, m) for m in range(ND)])
        if vtr_u is not None:
            # the transposes go several units after vmm so their LDWEIGHTS
            # never waits on the vtt copy queued behind exps
            units.insert(min(len(units), 10), vtr_u)
        filler = units

        nsb = 4 * tau + 4
        npair = nsb // 2
        total_steps = HPG * nsb
        step = 0
        fi = 0
        oproj_budget = [0, 99, 99, 99][tau]

        def try_fill():
            nonlocal fi, oproj_budget
            if fi >= len(filler):
                return
            u = filler[fi]
            if u[0] == "proj":
                u[1]()
                fi += 1
            elif oproj_budget > 0 and norms_done[u[1]] == HPG:
                oproj_block(u[1], u[2])
                fi += 1
                oproj_budget -= 1

        for j in range(HPG):
            h = {
                "otp": psot.tile([128, TW], F32, tag="ot", name="otp"),
                "ptsum2": ptsums.tile([128, 2, TW], BF16, tag="ps",
                                      name="ptsum2"),
                "pairs": {}, "tau": tau, "j": j, "npair": npair, "ncons": 0,
            }
            qslice = qt[j][:, tau * TW:(tau + 1) * TW]

            for sb in range(nsb):
                di = sb - 4 * tau
                lo = di * 128 if di >= 0 else 0   # valid t-range start
                if sb % 2 == 0:
                    stp = psst.tile([128, 2, TW], F32, tag="st")
                    ptq = ptp.tile([128, 2, TW], BF16, tag="pt")
                    h["pairs"][sb // 2] = [stp, ptq, lo, lo]
                pr = h["pairs"][sb // 2]
                pr[2 + sb % 2] = lo
                stp = pr[0]
                # plane 1 streams from the pair's lo0 so the batched pair
                # exp never touches bytes this tile didn't write; the
                # causally-invalid strip is masked out of pt after the exp
                slo = pr[2] if sb % 2 == 1 else lo
                nc.tensor.matmul(stp[:, sb % 2, slo:],
                                 kt[:, sb * 128:(sb + 1) * 128],
                                 qslice[:, slo:], start=True, stop=True)
                if sb % 2 == 1:
                    nc.scalar.activation(pr[1][:, :, pr[2]:],
                                         stp[:, :, pr[2]:], AF.Exp,
                                         scale=SCALE)
                    if di >= 0:
                        # causal triangle: zero pt's above-diagonal entries
                        # post-exp (keeps the DVE off the S->exp path).
                        # plane 1's mask is 256 wide ([zeros | tri]) so its
                        # [lo0, lo1) garbage strip reads as zero downstream
                        nc.vector.tensor_mul(
                            pr[1][:, 0, pr[2]:pr[2] + 128],
                            pr[1][:, 0, pr[2]:pr[2] + 128], tri01)
                        nc.vector.tensor_mul(
                            pr[1][:, 1, pr[2]:pr[3] + 128],
                            pr[1][:, 1, pr[2]:pr[3] + 128], tri256)
                    pend.append([h, sb // 2])
                    if len(pend) > 5:
                        consume_one()
                step += 1
                while fi < len(filler) and fi < (step * len(filler)) // total_steps:
                    n0 = fi
                    try_fill()
                    if fi == n0:
                        break
        # boundary: flush remaining proj units (flash(tau+1) needs them);
        # oproj units carry over into the next tau's list
        rest = filler[fi:]
        filler = []
        for u in rest:
            if u[0] == "proj":
                u[1]()
            else:
                filler.append(u)

    # drain: consume remaining pairs, finish normalizes, leftover oproj
    while pend:
        consume_one()
    norm_flush(drain=True)
    for t, m in [(u[1], u[2]) for u in filler]:
        oproj_block(t, m)
    # final oproj in waves of 4 (2 acc + 2 st psum chains): c0..2 of the
    # wave execute before head 3's normalize mul lands; the c3s + copies
    # follow, alternating ACT/DVE so the tail drains on two engines.
    for w0, wn in ((0, 4), (4, 4), (8, 4), (12, 2), (14, 2)):
        wave = []
        for i, m in enumerate(range(w0, w0 + wn)):
            pool = psacc if i < 2 else psst
            wave.append((m, oproj_start(NTAU - 1, m, pool)))
        for m, yp in wave:
            oproj_fin(NTAU - 1, m, yp, dve_copy=(m % 2 == 1))


def _build_nc():
    if "nc" in _CACHE:
        return _CACHE["nc"]
    nc = bacc.Bacc("TRN2", target_bir_lowering=False, debug=False)
    xT = nc.dram_tensor("xT", [D, T], BF16, kind="ExternalInput").ap()
    wq = nc.dram_tensor("wq", [D, QC], BF16, kind="ExternalInput").ap()
    wk = nc.dram_tensor("wk", [D, DH], BF16, kind="ExternalInput").ap()
    wv = nc.dram_tensor("wv", [D, DH], BF16, kind="ExternalInput").ap()
    wo = nc.dram_tensor("wo", [QC, D], BF16, kind="ExternalInput").ap()
    bq = nc.dram_tensor("bq", [QC], F32, kind="ExternalInput").ap()
    bk = nc.dram_tensor("bk", [DH], F32, kind="ExternalInput").ap()
    maskTd = nc.dram_tensor("maskT", [128, 128], BF16, kind="ExternalInput").ap()
    maskT2d = nc.dram_tensor("maskT2", [128, 256], BF16, kind="ExternalInput").ap()
    identd = nc.dram_tensor("ident", [128, 128], BF16, kind="ExternalInput").ap()
    yT = nc.dram_tensor("yT", [D, T], BF16, kind="ExternalOutput").ap()

    with tile.TileContext(nc) as tc, ExitStack() as ctx:
        _body(ctx, tc, xT, wq, wk, wv, wo, bq, bk, maskTd, maskT2d, identd, yT)
    nc.compile()
    _CACHE["nc"] = nc
    return nc


def _host_consts():
    p = np.arange(128)[:, None]
    f = np.arange(128)[None, :]
    maskT = np.where(f >= p, 1.0, 0.0).astype(ml_dtypes.bfloat16)
    maskT2 = np.concatenate(
        [np.zeros((128, 128), ml_dtypes.bfloat16), maskT], axis=1)
    maskT2 = np.ascontiguousarray(maskT2)
    ident = np.eye(128, dtype=ml_dtypes.bfloat16)
    return maskT, maskT2, ident


def make_in_maps(x, Wq, bq, Wk, bk, Wv, bv, Wo, bo):
    maskT, maskT2, ident = _host_consts()
    bf = lambda a: np.ascontiguousarray(a).astype(ml_dtypes.bfloat16)

    xTb = [bf(x[b].T) for b in range(2)]
    in_maps = []
    for c in range(8):
        b, g = divmod(c, G)
        in_maps.append({
            "xT": xTb[b],
            "wq": bf(Wq[:, g * QC:(g + 1) * QC]),
            "wk": bf(Wk[:, g * DH:(g + 1) * DH]),
            "wv": bf(Wv[:, g * DH:(g + 1) * DH]),
            "wo": bf(Wo[g * QC:(g + 1) * QC, :]),
            "bq": np.ascontiguousarray(bq[g * QC:(g + 1) * QC]),
            "bk": np.ascontiguousarray(bk[g * DH:(g + 1) * DH]),
            "maskT": maskT,
            "maskT2": maskT2,
            "ident": ident,
        })
    return in_maps


def kernel(x, Wq, bq, Wk, bk, Wv, bv, Wo, bo):
    global LAST_RESULTS
    x = np.asarray(x, np.float32)
    Wq = np.asarray(Wq, np.float32)
    Wk = np.asarray(Wk, np.float32)
    Wv = np.asarray(Wv, np.float32)
    Wo = np.asarray(Wo, np.float32)
    bq = np.asarray(bq, np.float32)
    bk = np.asarray(bk, np.float32)
    bv = np.asarray(bv, np.float32)
    bo = np.asarray(bo, np.float32)

    nc = _build_nc()
    in_maps = make_in_maps(x, Wq, bq, Wk, bk, Wv, bv, Wo, bo)

    res = run_bass_kernel_spmd(nc, in_maps, list(range(8)), trace=TRACE,
                               **TRACE_KW)
    LAST_RESULTS = res

    # V bias folded: bo_eff = bo + (bv per head) @ Wo
    bv_heads = np.repeat(bv.reshape(G, DH), HPG, axis=0).reshape(-1)
    bo_eff = bo + bv_heads @ Wo

    y = np.empty((2, T, D), np.float32)
    for b in range(2):
        acc = res.results[b * G + 0]["yT"].astype(np.float32)
        for g in range(1, G):
            acc += res.results[b * G + g]["yT"].astype(np.float32)
        y[b] = acc.T + bo_eff
    return y


# revision 63
# speedup vs baseline: 1.0694x; 1.0062x over previous
"""GroupedQueryAttention Trainium2 Bass kernel (v6).

Sharding: 8 cores = (B=2) x (G=4 KV groups). Each core computes, for its
(batch b, kv-group g): the 4 query heads' Q/K/V projections, causal flash
attention, and a partial output projection Y^T_g (bf16). Host sums the 4
partials per batch and adds an adjusted bias (bo + bv-term folded in).

Key structure (all transposed: token dim T on the free axis):
  xT[d, t]     uploaded pre-transposed from host (bf16)
  Q^T, K^T     from projection matmuls (W chunk stationary, xT moving)
  V^T -> V     PE transpose per 128-block, staged in the st PSUM slots
  S^T[s, t]  = (K^T s-block).T @ Q^T        (one 128-wide matmul per s-block)
  P^T        = exp(scale * S^T + mask)      (ACT, PSUM -> SBUF, bf16)
  O^T[dh, t] += (V s-block).T @ P^T         (PSUM accumulation over s-blocks)
  rowsum     += ones.T-style P sums (DVE bf16 adds per pair)
  Y^T[dm, t] = sum_c (Wo chunk).T @ O^T_c   (per 128-row dm block, bf16 out)

The V bias never enters the kernel: O = (P@(V0+1*bv))/rowsum = P@V0/rowsum
+ bv, and the constant bv contribution to Y is folded into bo on the host.

v6 scheduling (on the v2/v3 spine):
  - Unified filler stream: flash(tau) absorbs oproj(tau-1) m-blocks AND
    the K/V/Q projection chains of tau+1, so the tau boundary has no proj
    bubble. The V chain is split into two units - matmuls+copy early, PE
    transposes several steps later - so the transpose LDWEIGHTS never
    parks the PE FIFO on the ACT queue.
  - qt/kt PSUM->SBUF bias-copies on DVE (tensor_scalar_add), off the ACT
    FIFO that feeds exp.
  - pend FIFO depth 3 (PV consumes 3 pairs behind the S/exp front) to ride
    out exp latency spikes.
  - Light oproj budgets shift some m-blocks from the PE-bound flash(1/2)
    windows into the exp-bound flash(3); yT stores alternate the sync and
    gpsimd queues, and tail ys copies alternate ACT/DVE, so the drain ends
    within ~2us of the last matmul.

Normalize chain: ptsum adds (DVE, bf16) -> gpsimd 128-way all-reduce (f32)
-> reciprocal_approx_fast (DVE) -> mul (DVE), recip+mul deferred a few
consume slots so the DVE FIFO never blocks on the all-reduce.
"""

import sys

sys.path.insert(0, "/opt/trn_rl_repo")

from contextlib import ExitStack

import ml_dtypes
import numpy as np

import concourse.bass as bass  # noqa: F401
import concourse.tile as tile
from concourse import bacc, bass_isa, mybir
from concourse.bass_utils import run_bass_kernel_spmd

F32 = mybir.dt.float32
BF16 = mybir.dt.bfloat16
AF = mybir.ActivationFunctionType

D = 2048          # model dim
T = 2048          # tokens
DH = 128          # head dim
G = 4             # kv groups
HPG = 4           # query heads per group
QC = HPG * DH     # query cols per group = 512
ND = D // 128     # 16 contraction chunks
NTAU = 4          # t tiles of 512
TW = 512          # t tile width
SCALE = DH ** -0.5

TRACE = False
TRACE_KW = {}
LAST_RESULTS = None

_CACHE = {}


def _body(ctx, tc, xT, wq, wk, wv, wo, bq, bk, maskTd, maskT2d, identd, yT):
    nc = tc.nc

    # PSUM (16KB/partition exactly): acc 2x2KB + st-pair 2x4KB (shared with
    # V-transpose staging) + ot 2x2KB
    psacc = ctx.enter_context(tc.tile_pool(name="psacc", bufs=2, space="PSUM"))
    psst = ctx.enter_context(tc.tile_pool(name="psst", bufs=2, space="PSUM"))
    psot = ctx.enter_context(tc.tile_pool(name="psot", bufs=2, space="PSUM"))

    consts = ctx.enter_context(tc.tile_pool(name="consts", bufs=1))
    qkv = ctx.enter_context(tc.tile_pool(name="qkv", bufs=1))
    xtp = ctx.enter_context(tc.tile_pool(name="xtp", bufs=ND))
    wkp = ctx.enter_context(tc.tile_pool(name="wkp", bufs=ND))
    wvp = ctx.enter_context(tc.tile_pool(name="wvp", bufs=ND))
    wqp = ctx.enter_context(tc.tile_pool(name="wqp", bufs=ND))
    wop = ctx.enter_context(tc.tile_pool(name="wop", bufs=1))
    vts = ctx.enter_context(tc.tile_pool(name="vstage", bufs=2))
    # pend depth 5 keeps 5 unconsumed P tiles + 1 being written in flight
    ptp = ctx.enter_context(tc.tile_pool(name="ptp", bufs=6))
    ptsums = ctx.enter_context(tc.tile_pool(name="ptsums", bufs=2))
    nrm = ctx.enter_context(tc.tile_pool(name="norm", bufs=2))
    otp_pool = ctx.enter_context(tc.tile_pool(name="otsb", bufs=1))
    yb = ctx.enter_context(tc.tile_pool(name="ybounce", bufs=3))

    # ---- constants on the scalar queue (small, early)
    tri01 = consts.tile([128, 128], BF16, tag="tri01")
    nc.scalar.dma_start(tri01, maskTd)
    tri256 = consts.tile([128, 256], BF16, tag="tri256")
    nc.scalar.dma_start(tri256, maskT2d)
    bqt = consts.tile([128, 4], F32, tag="bqt")
    nc.scalar.dma_start(bqt, bq.rearrange("(c p) -> p c", p=128))
    bkt = consts.tile([128, 1], F32, tag="bkt")
    nc.scalar.dma_start(bkt, bk.rearrange("(c p) -> p c", p=128))
    ident = consts.tile([128, 128], BF16, tag="ident")
    nc.scalar.dma_start(ident, identd)

    # ---- weights + x on the two fast queues (sync HWDGE, gpsimd SWDGE),
    # strictly in first-use order: wk, x(sg0), wv, wq, x(sg1..3), wo.
    xts = [xtp.tile([128, T], BF16, tag="xt", name=f"xt{d}") for d in range(ND)]
    wkts = [wkp.tile([128, DH], BF16, tag="wk", name=f"wk{d}") for d in range(ND)]
    wvts = [wvp.tile([128, DH], BF16, tag="wv", name=f"wv{d}") for d in range(ND)]
    wqts = [wqp.tile([128, QC], BF16, tag="wq", name=f"wq{d}") for d in range(ND)]
    wot = [wop.tile([128, D], BF16, tag=f"wo{c}", name=f"wo{c}") for c in range(HPG)]

    qlist = [nc.sync, nc.gpsimd]
    qi = 0

    def q_next():
        nonlocal qi
        eng = qlist[qi % 2]
        qi += 1
        return eng

    for d in range(ND):
        q_next().dma_start(wkts[d], wk[d * 128:(d + 1) * 128, :])
        q_next().dma_start(xts[d][:, 0:TW], xT[d * 128:(d + 1) * 128, 0:TW])
    for d in range(ND):
        q_next().dma_start(wvts[d], wv[d * 128:(d + 1) * 128, :])
    for d in range(ND):
        q_next().dma_start(wqts[d], wq[d * 128:(d + 1) * 128, :])
    for sg in range(1, NTAU):
        for d in range(ND):
            q_next().dma_start(
                xts[d][:, sg * TW:(sg + 1) * TW],
                xT[d * 128:(d + 1) * 128, sg * TW:(sg + 1) * TW])
    for c in range(HPG):
        q_next().dma_start(wot[c], wo[c * 128:(c + 1) * 128, :])

    # ---- HAM warm-up: real matmuls on a memset tile (no DMA dependency)
    # while the x DMAs land
    warm_in = consts.tile([128, 128], BF16, tag="warm_in")
    nc.vector.memset(warm_in, 0.0)

    def warm_fill(n):
        for w in range(n):
            wps = psot.tile([128, 128], F32, tag="ot", name="warm")
            nc.tensor.matmul(wps, warm_in, warm_in, start=True, stop=True)

    warm_fill(72)

    qt = [qkv.tile([128, T], BF16, tag=f"qt{j}", name=f"qt{j}") for j in range(HPG)]
    kt = qkv.tile([128, T], BF16, tag="kt")
    vv = qkv.tile([128, ND, 128], BF16, tag="vv")  # [s%128, s_block, dh]

    # ---- K projection chain for one sg column block (kt copy on DVE)
    def kchain(sg):
        ps = psacc.tile([128, TW], F32, tag="acc", name="psk")
        for d in range(ND):
            nc.tensor.matmul(ps, wkts[d], xts[d][:, sg * TW:(sg + 1) * TW],
                             start=(d == 0), stop=(d == ND - 1))
        nc.vector.tensor_scalar_add(kt[:, sg * TW:(sg + 1) * TW], ps,
                                    bkt[:, 0:1])

    # ---- V projection, split in two units: matmuls + vtt copy first, the
    # PE transposes several filler steps later so their LDWEIGHTS never
    # waits on the ACT queue while parking the PE FIFO
    vstash = {}

    def vmm(sg):
        ps2 = psacc.tile([128, TW], F32, tag="acc", name="psv")
        for d in range(ND):
            nc.tensor.matmul(ps2, wvts[d], xts[d][:, sg * TW:(sg + 1) * TW],
                             start=(d == 0), stop=(d == ND - 1))
        vtt = vts.tile([128, TW], BF16, tag="vt")
        nc.scalar.copy(vtt, ps2)
        vstash[sg] = vtt

    def vtr(sg):
        vtt = vstash.pop(sg)
        stg = psst.tile([128, TW], BF16, tag="st", name="vstg")
        for i in range(4):
            nc.tensor.transpose(stg[:, i * 128:(i + 1) * 128],
                                vtt[:, i * 128:(i + 1) * 128], ident)
        nc.vector.tensor_copy(vv[:, sg * 4:(sg + 1) * 4, :], stg)

    # ---- Q projection chain for one (t-tile, head block) (qt copy on DVE)
    def qchain(tau, cb):
        ps = psacc.tile([128, TW], F32, tag="acc", name="psq")
        for d in range(ND):
            nc.tensor.matmul(
                ps, wqts[d][:, cb * 128:(cb + 1) * 128],
                xts[d][:, tau * TW:(tau + 1) * TW],
                start=(d == 0), stop=(d == ND - 1))
        nc.vector.tensor_scalar_add(qt[cb][:, tau * TW:(tau + 1) * TW], ps,
                                    bqt[:, cb:cb + 1])

    # ---- output projection m-block, split emission: c0..2 accumulate,
    # then c3 + copy + store
    ots = [otp_pool.tile([128, T], BF16, tag=f"ot{j}", name=f"ots{j}")
           for j in range(HPG)]

    def oproj_start(tau, m, pool):
        if pool is psst:
            big = pool.tile([128, 2, TW], F32, tag="st", name="ypst")
            yp = big[:, 0, :]
        else:
            yp = pool.tile([128, TW], F32, tag="acc", name="yp")
        for c in range(HPG - 1):
            nc.tensor.matmul(
                yp, wot[c][:, m * 128:(m + 1) * 128],
                ots[c][:, tau * TW:(tau + 1) * TW],
                start=(c == 0), stop=False)
        return yp

    def oproj_fin(tau, m, yp, dve_copy=False):
        c = HPG - 1
        nc.tensor.matmul(
            yp, wot[c][:, m * 128:(m + 1) * 128],
            ots[c][:, tau * TW:(tau + 1) * TW],
            start=False, stop=True)
        ys = yb.tile([128, TW], BF16, tag="y", name="ys")
        if dve_copy:
            nc.vector.tensor_copy(ys, yp)
        else:
            nc.scalar.copy(ys, yp)
        # all stores on the sync HWDGE queue: gpsimd SWDGE triggers are
        # ~660ns each and serialize the drain tail
        nc.sync.dma_start(
            yT[m * 128:(m + 1) * 128, tau * TW:(tau + 1) * TW], ys)

    def oproj_block(tau, m):
        yp = oproj_start(tau, m, psacc)
        oproj_fin(tau, m, yp, dve_copy=(m % 2 == 1))

    # ---- phase C: per-tau pipeline. flash(tau) spine = S -> exp -> PV with
    # the pend FIFO 3 pairs behind; fillers = proj chains of tau+1 +
    # budgeted oproj(tau-1) m-blocks, paced across the spine steps.

    pend = []
    norm_pend = []
    cc = [0]
    norms_done = [0] * NTAU

    def norm_flush(drain=False):
        while norm_pend and (drain or norm_pend[0][2] <= cc[0]):
            h, rc, _ = norm_pend.pop(0)
            rcr = nrm.tile([128, TW], F32, tag="rcr")
            nc.vector.reciprocal_approx_fast(rcr, rc)
            nc.vector.tensor_mul(
                ots[h["j"]][:, h["tau"] * TW:(h["tau"] + 1) * TW],
                h["otp"], rcr)
            norms_done[h["tau"]] += 1

    def consume_one():
        cc[0] += 1
        norm_flush()
        h, q = pend.pop(0)
        otp, ptsum2, pairs, npair_ = (
            h["otp"], h["ptsum2"], h["pairs"], h["npair"])
        _, ptq, lo0, lo1 = pairs[q]
        first = h["ncons"] == 0
        h["ncons"] += 1
        last = h["ncons"] == npair_
        nc.tensor.matmul(otp[:, lo0:], vv[:, 2 * q, :],
                         ptq[:, 0, lo0:], start=first, stop=False)
        nc.tensor.matmul(otp[:, lo1:], vv[:, 2 * q + 1, :],
                         ptq[:, 1, lo1:], start=False, stop=last)
        # denominator: both planes' per-partition P-sums accumulate in one
        # DVE op per pair (plane 1's [lo0, lo1) strip is mask-zeroed, so
        # the full-width add is garbage-free); folded at the head's end
        if first:
            nc.vector.tensor_copy(ptsum2, ptq)
        else:
            nc.vector.tensor_add(ptsum2[:, :, lo0:], ptsum2[:, :, lo0:],
                                 ptq[:, :, lo0:])
        del pairs[q]
        if last:
            rs = ptsums.tile([128, TW], BF16, tag="rsf", name="rsf")
            nc.vector.tensor_add(rs, ptsum2[:, 0, :], ptsum2[:, 1, :])
            rc = nrm.tile([128, TW], F32, tag="rc")
            nc.gpsimd.partition_all_reduce(rc, rs, 128,
                                           bass_isa.ReduceOp.add)
            # defer the DVE recip+mul 4 consume slots (~3.6us): the gpsimd
            # all-reduce takes ~3.5us and an early recip would park the DVE
            # FIFO (and the masks behind it) on that wait
            norm_pend.append([h, rc, cc[0] + 4])

    # prologue: projections for tau 0 run immediately
    kchain(0)
    vmm(0)
    for cb in range(HPG):
        qchain(0, cb)
    vtr(0)

    # filler units: ("proj", fn) always eligible; ("oproj", t, m) eligible
    # once norms_done[t] == HPG, capped per tau by the oproj budget.
    filler = []
    for tau in range(NTAU):
        units = []
        vtr_u = None
        if tau + 1 < NTAU:
            units.append(("proj", lambda sg=tau + 1: kchain(sg)))
            units.append(("proj", lambda sg=tau + 1: vmm(sg)))
            for cb in range(HPG):
                units.append(("proj", lambda sg=tau + 1, cb=cb: qchain(sg, cb)))
            vtr_u = ("proj", lambda sg=tau + 1: vtr(sg))
        units.extend(filler)
        if tau > 0:
            units.extend([("oproj", tau - 1, m) for m in range(ND)])
        if vtr_u is not None:
            # the transposes go several units after vmm so their LDWEIGHTS
            # never waits on the vtt copy queued behind exps
            units.insert(min(len(units), 10), vtr_u)
        filler = units

        nsb = 4 * tau + 4
        npair = nsb // 2
        total_steps = HPG * nsb
        step = 0
        fi = 0
        oproj_budget = [0, 99, 12, 99][tau]

        def try_fill():
            nonlocal fi, oproj_budget
            if fi >= len(filler):
                return
            u = filler[fi]
            if u[0] == "proj":
                u[1]()
                fi += 1
            elif oproj_budget > 0 and norms_done[u[1]] == HPG:
                oproj_block(u[1], u[2])
                fi += 1
                oproj_budget -= 1

        for j in range(HPG):
            h = {
                "otp": psot.tile([128, TW], F32, tag="ot", name="otp"),
                "ptsum2": ptsums.tile([128, 2, TW], BF16, tag="ps",
                                      name="ptsum2"),
                "pairs": {}, "tau": tau, "j": j, "npair": npair, "ncons": 0,
            }
            qslice = qt[j][:, tau * TW:(tau + 1) * TW]

            for sb in range(nsb):
                di = sb - 4 * tau
                lo = di * 128 if di >= 0 else 0   # valid t-range start
                if sb % 2 == 0:
                    stp = psst.tile([128, 2, TW], F32, tag="st")
                    ptq = ptp.tile([128, 2, TW], BF16, tag="pt")
                    h["pairs"][sb // 2] = [stp, ptq, lo, lo]
                pr = h["pairs"][sb // 2]
                pr[2 + sb % 2] = lo
                stp = pr[0]
                # plane 1 streams from the pair's lo0 so the batched pair
                # exp never touches bytes this tile didn't write; the
                # causally-invalid strip is masked out of pt after the exp
                slo = pr[2] if sb % 2 == 1 else lo
                nc.tensor.matmul(stp[:, sb % 2, slo:],
                                 kt[:, sb * 128:(sb + 1) * 128],
                                 qslice[:, slo:], start=True, stop=True)
                if sb % 2 == 1:
                    nc.scalar.activation(pr[1][:, :, pr[2]:],
                                         stp[:, :, pr[2]:], AF.Exp,
                                         scale=SCALE)
                    if di >= 0:
                        # causal triangle: zero pt's above-diagonal entries
                        # post-exp (keeps the DVE off the S->exp path).
                        # plane 1's mask is 256 wide ([zeros | tri]) so its
                        # [lo0, lo1) garbage strip reads as zero downstream
                        nc.vector.tensor_mul(
                            pr[1][:, 0, pr[2]:pr[2] + 128],
                            pr[1][:, 0, pr[2]:pr[2] + 128], tri01)
                        nc.vector.tensor_mul(
                            pr[1][:, 1, pr[2]:pr[3] + 128],
                            pr[1][:, 1, pr[2]:pr[3] + 128], tri256)
                    pend.append([h, sb // 2])
                    if len(pend) > 5:
                        consume_one()
                step += 1
                while fi < len(filler) and fi < (step * len(filler)) // total_steps:
                    n0 = fi
                    try_fill()
                    if fi == n0:
                        break
        # boundary: flush remaining proj units (flash(tau+1) needs them);
        # oproj units carry over into the next tau's list
        rest = filler[fi:]
        filler = []
        for u in rest:
            if u[0] == "proj":
                u[1]()
            else:
                filler.append(u)

    # drain: consume remaining pairs, finish normalizes, leftover oproj
    while pend:
        consume_one()
    norm_flush(drain=True)
    for t, m in [(u[1], u[2]) for u in filler]:
        oproj_block(t, m)
    # final oproj in waves of 4 (2 acc + 2 st psum chains): c0..2 of the
    # wave execute before head 3's normalize mul lands; the c3s + copies
    # follow, alternating ACT/DVE so the tail drains on two engines.
    for w0, wn in ((0, 4), (4, 4), (8, 4), (12, 2), (14, 2)):
        wave = []
        for i, m in enumerate(range(w0, w0 + wn)):
            pool = psacc if i < 2 else psst
            wave.append((m, oproj_start(NTAU - 1, m, pool)))
        for m, yp in wave:
            oproj_fin(NTAU - 1, m, yp, dve_copy=(m % 2 == 1))


def _build_nc():
    if "nc" in _CACHE:
        return _CACHE["nc"]
    nc = bacc.Bacc("TRN2", target_bir_lowering=False, debug=False)
    xT = nc.dram_tensor("xT", [D, T], BF16, kind="ExternalInput").ap()
    wq = nc.dram_tensor("wq", [D, QC], BF16, kind="ExternalInput").ap()
    wk = nc.dram_tensor("wk", [D, DH], BF16, kind="ExternalInput").ap()
    wv = nc.dram_tensor("wv", [D, DH], BF16, kind="ExternalInput").ap()
    wo = nc.dram_tensor("wo", [QC, D], BF16, kind="ExternalInput").ap()
    bq = nc.dram_tensor("bq", [QC], F32, kind="ExternalInput").ap()
    bk = nc.dram_tensor("bk", [DH], F32, kind="ExternalInput").ap()
    maskTd = nc.dram_tensor("maskT", [128, 128], BF16, kind="ExternalInput").ap()
    maskT2d = nc.dram_tensor("maskT2", [128, 256], BF16, kind="ExternalInput").ap()
    identd = nc.dram_tensor("ident", [128, 128], BF16, kind="ExternalInput").ap()
    yT = nc.dram_tensor("yT", [D, T], BF16, kind="ExternalOutput").ap()

    with tile.TileContext(nc) as tc, ExitStack() as ctx:
        _body(ctx, tc, xT, wq, wk, wv, wo, bq, bk, maskTd, maskT2d, identd, yT)
    nc.compile()
    _CACHE["nc"] = nc
    return nc


def _host_consts():
    p = np.arange(128)[:, None]
    f = np.arange(128)[None, :]
    maskT = np.where(f >= p, 1.0, 0.0).astype(ml_dtypes.bfloat16)
    maskT2 = np.concatenate(
        [np.zeros((128, 128), ml_dtypes.bfloat16), maskT], axis=1)
    maskT2 = np.ascontiguousarray(maskT2)
    ident = np.eye(128, dtype=ml_dtypes.bfloat16)
    return maskT, maskT2, ident


def make_in_maps(x, Wq, bq, Wk, bk, Wv, bv, Wo, bo):
    maskT, maskT2, ident = _host_consts()
    bf = lambda a: np.ascontiguousarray(a).astype(ml_dtypes.bfloat16)

    xTb = [bf(x[b].T) for b in range(2)]
    in_maps = []
    for c in range(8):
        b, g = divmod(c, G)
        in_maps.append({
            "xT": xTb[b],
            "wq": bf(Wq[:, g * QC:(g + 1) * QC]),
            "wk": bf(Wk[:, g * DH:(g + 1) * DH]),
            "wv": bf(Wv[:, g * DH:(g + 1) * DH]),
            "wo": bf(Wo[g * QC:(g + 1) * QC, :]),
            "bq": np.ascontiguousarray(bq[g * QC:(g + 1) * QC]),
            "bk": np.ascontiguousarray(bk[g * DH:(g + 1) * DH]),
            "maskT": maskT,
            "maskT2": maskT2,
            "ident": ident,
        })
    return in_maps


def kernel(x, Wq, bq, Wk, bk, Wv, bv, Wo, bo):
    global LAST_RESULTS
    x = np.asarray(x, np.float32)
    Wq = np.asarray(Wq, np.float32)
    Wk = np.asarray(Wk, np.float32)
    Wv = np.asarray(Wv, np.float32)
    Wo = np.asarray(Wo, np.float32)
    bq = np.asarray(bq, np.float32)
    bk = np.asarray(bk, np.float32)
    bv = np.asarray(bv, np.float32)
    bo = np.asarray(bo, np.float32)

    nc = _build_nc()
    in_maps = make_in_maps(x, Wq, bq, Wk, bk, Wv, bv, Wo, bo)

    res = run_bass_kernel_spmd(nc, in_maps, list(range(8)), trace=TRACE,
                               **TRACE_KW)
    LAST_RESULTS = res

    # V bias folded: bo_eff = bo + (bv per head) @ Wo
    bv_heads = np.repeat(bv.reshape(G, DH), HPG, axis=0).reshape(-1)
    bo_eff = bo + bv_heads @ Wo

    y = np.empty((2, T, D), np.float32)
    for b in range(2):
        acc = res.results[b * G + 0]["yT"].astype(np.float32)
        for g in range(1, G):
            acc += res.results[b * G + g]["yT"].astype(np.float32)
        y[b] = acc.T + bo_eff
    return y
